# revision 37
# baseline (speedup 1.0000x reference)
"""HGT (heterogeneous graph transformer) on 8 TRN2 NeuronCores.

Single-launch, fully on-device design:
  - Nodes type-sorted, dealt round-robin to 8 cores (SPMD: one NEFF).
  - Device: adapt (tanh per-type linear) -> per layer: per-type K|V|Q table
    matmul for owned nodes -> AllGather table (bf16) -> edge phase on device
    (dma_gather of K/V rows by src, RTE rows by (type,time), Q rows by dst;
    per-relation block-diag matmuls; unstabilized segment softmax via exp +
    scatter-added denominators; selection-matrix merge per 128-edge tile;
    dma_scatter_add into aggr table) -> AllReduce aggr -> normalize / gelu /
    per-type update matmul -> residual mix, all in SBUF-resident xT.
  - Edges sorted by (edge_type, src_half, dst_half, dst_row): 16 groups.
    Halves solve the int16 gather-index range; relation grouping gives
    sliced block-diag matmuls; dst sort + per-tile leader tokens give
    collision-free scatter-adds.
  - Host: only integer index prep + bf16 packing + final transpose.
"""
import sys, math, os, time
sys.path.insert(0, '/opt/trn_rl_repo')
import numpy as np
import ml_dtypes

import concourse.bass as bass
import concourse.tile as tile
from concourse import bacc, mybir
from concourse import bass_utils

BF16 = mybir.dt.bfloat16
F32 = mybir.dt.float32
I16 = mybir.dt.int16
I32 = mybir.dt.int32
NCORES = 8
N, E = 50000, 300000
IN_DIM, NH, HEADS, DK = 256, 128, 8, 16
T, R, L = 3, 4, 2
SQRT_DK = math.sqrt(DK)
HALF = 32768
BLK = 512

_T0 = time.time()
def _tick(msg):
    print(f"[kernel +{time.time()-_T0:7.2f}s] {msg}", file=sys.stderr, flush=True)

_cache = {}


def build_neff(LNP, sections, P_gs, NT, NCH, WSHARD, debug=False):
    """One SPMD NEFF for the whole model.

    sections: list of (type, c0, c1) node ranges (multiples of 128).
    P_gs: list of 16 per-group padded edge counts (multiples of BLK).
    NT = sum(P_gs)//128 tile count; NCH = LNP//128 node chunk count.
    """
    RT = NCORES * LNP
    EP = sum(P_gs)
    nc = bacc.Bacc("TRN2", target_bir_lowering=False, debug=False,
                   num_devices=NCORES)
    # ---- inputs ----
    featT = nc.dram_tensor("featT", [IN_DIM, LNP], BF16, kind="ExternalInput").ap()
    wsh_d = nc.dram_tensor("wblob", [WSHARD], BF16, kind="ExternalInput").ap()
    ab_d = nc.dram_tensor("ab", [NH, T], F32, kind="ExternalInput").ap()
    abl_d = nc.dram_tensor("abl", [NH, L * T], F32, kind="ExternalInput").ap()
    headT_d = nc.dram_tensor("headT", [NH, HEADS], F32, kind="ExternalInput").ap()
    headb_d = nc.dram_tensor("headb", [HEADS, NH], F32, kind="ExternalInput").ap()
    ones1_d = nc.dram_tensor("ones1", [1, NH], BF16, kind="ExternalInput").ap()
    identf_d = nc.dram_tensor("identf", [128, 128], F32, kind="ExternalInput").ap()
    wblob = nc.dram_tensor("wblob_all", [NCORES * WSHARD], BF16,
                           kind="Internal", addr_space="Shared").ap()
    wstage = nc.dram_tensor("wstage", [WSHARD], BF16, kind="Internal").ap()

    def wv(off, rows, cols):
        return wblob[off:off + rows * cols].rearrange("(a b) -> a b", a=rows)
    kvidx_d = nc.dram_tensor("kvidx", [16, EP // 16], I16, kind="ExternalInput").ap()
    rteidx_d = nc.dram_tensor("rteidx", [16, EP // 16], I16, kind="ExternalInput").ap()
    qidx_d = nc.dram_tensor("qidx", [16, EP // 16], I16, kind="ExternalInput").ap()
    scidx_d = nc.dram_tensor("scidx", [16, EP // 16], I16, kind="ExternalInput").ap()
    dstid_d = nc.dram_tensor("dstid", [128, NT], F32, kind="ExternalInput").ap()
    updidx_d = nc.dram_tensor("updidx", [128, NCH], I32, kind="ExternalInput").ap()
    # ---- output ----
    xout = nc.dram_tensor("xout", [NH, LNP], BF16, kind="ExternalOutput").ap()
    if debug:
        xa_out = nc.dram_tensor("xa_out", [NH, LNP], BF16, kind="ExternalOutput").ap()
        tab_out = nc.dram_tensor("tab_out", [NCORES * LNP, 3 * NH], BF16, kind="ExternalOutput").ap()
        ag_out = nc.dram_tensor("ag_out", [NCORES * LNP, 192], F32, kind="ExternalOutput").ap()
    # ---- internal DRAM ----
    tab_own = nc.dram_tensor("tab_own", [LNP, 3 * NH], BF16, kind="Internal").ap()
    tab_all = nc.dram_tensor("tab_all", [RT, 3 * NH], BF16, kind="Internal",
                             addr_space="Shared").ap()
    aggr = nc.dram_tensor("aggr", [RT, 192], F32, kind="Internal").ap()
    aggr2 = nc.dram_tensor("aggr2", [RT, 192], F32, kind="Internal",
                           addr_space="Shared").ap()

    sec_of = {}
    for (t, c0, c1) in sections:
        for c in range(c0, c1, 128):
            sec_of[c] = t

    groups = []  # (et, sh, dh, P_g)
    gi = 0
    for et in range(R):
        for sh in range(2):
            for dh in range(2):
                groups.append((et, sh, dh, P_gs[gi]))
                gi += 1

    with tile.TileContext(nc) as tc:
        with tc.tile_pool(name="persist", bufs=1) as pp, \
             tc.tile_pool(name="wts", bufs=1) as wp, \
             tc.tile_pool(name="io", bufs=3) as iop, \
             tc.tile_pool(name="edge", bufs=2) as ep, \
             tc.tile_pool(name="tilework", bufs=2) as twp:
            # ================= persistent loads =================
            nc.sync.dma_start(wstage[:], wsh_d[:])
            nc.gpsimd.collective_compute(
                "AllGather", mybir.AluOpType.bypass,
                replica_groups=[list(range(NCORES))],
                ins=[wstage[:]], outs=[wblob[:]])
            xT = pp.tile([128, LNP], BF16, tag="xT")
            kvidx = pp.tile([128, EP // 16], I16, tag="kvidx")
            rteidx = pp.tile([128, EP // 16], I16, tag="rteidx")
            qidx = pp.tile([128, EP // 16], I16, tag="qidx")
            scidx = pp.tile([128, EP // 16], I16, tag="scidx")
            for (sb, d) in ((kvidx, kvidx_d), (rteidx, rteidx_d),
                            (qidx, qidx_d), (scidx, scidx_d)):
                for k in range(8):
                    nc.sync.dma_start(sb[16 * k:16 * (k + 1), :], d[:, :])
            dstid = pp.tile([128, NT], F32, tag="dstid")
            nc.sync.dma_start(dstid[:], dstid_d[:, :])
            updidx = pp.tile([128, NCH], I32, tag="updidx")
            nc.sync.dma_start(updidx[:], updidx_d[:, :])
            headT = pp.tile([128, HEADS], F32, tag="headT")
            nc.sync.dma_start(headT[:], headT_d[:, :])
            headb = pp.tile([HEADS, 128], F32, tag="headb")
            nc.sync.dma_start(headb[:], headb_d[:, :])
            ones1 = pp.tile([1, 128], BF16, tag="ones1")
            nc.sync.dma_start(ones1[:], ones1_d[:, :])
            identf = pp.tile([128, 128], F32, tag="identf")
            nc.sync.dma_start(identf[:], identf_d[:, :])
            zero192 = pp.tile([128, 192], F32, tag="zero192")
            nc.vector.memset(zero192[:], 0.0)
            # weights (from allgathered blob; offsets match host pack order)
            cur = [0]

            def take(rows, cols):
                v = wv(cur[0], rows, cols)
                cur[0] += rows * cols
                return v

            aw_sb = wp.tile([128, T * 2 * NH], BF16, tag="aw")
            for t in range(T):
                for k in range(2):
                    nc.sync.dma_start(
                        aw_sb[:, (t * 2 + k) * NH:(t * 2 + k + 1) * NH],
                        take(128, NH))
            wkvq_sb = wp.tile([128, L * T * 3 * NH], BF16, tag="wkvq")
            for l in range(L):
                for t in range(T):
                    nc.sync.dma_start(
                        wkvq_sb[:, (l * T + t) * 3 * NH:(l * T + t + 1) * 3 * NH],
                        take(NH, 3 * NH))
            bkvq_sb = wp.tile([1, L * T * 3 * NH], BF16, tag="bkvq")
            for i in range(L * T):
                nc.sync.dma_start(bkvq_sb[:, i * 3 * NH:(i + 1) * 3 * NH],
                                  take(1, 3 * NH))
            bdatt_sb = wp.tile([128, L * R * NH], BF16, tag="bdatt")
            bdmsg_sb = wp.tile([128, L * R * NH], BF16, tag="bdmsg")
            for l in range(L):
                for r in range(R):
                    nc.sync.dma_start(
                        bdatt_sb[:, (l * R + r) * NH:(l * R + r + 1) * NH],
                        take(NH, NH))
                    nc.sync.dma_start(
                        bdmsg_sb[:, (l * R + r) * NH:(l * R + r + 1) * NH],
                        take(NH, NH))
            awl_sb = wp.tile([128, L * T * NH], BF16, tag="awl")
            for l in range(L):
                for t in range(T):
                    nc.sync.dma_start(
                        awl_sb[:, (l * T + t) * NH:(l * T + t + 1) * NH],
                        take(NH, NH))
            identb = pp.tile([128, 128], BF16, tag="identb")
            nc.sync.dma_start(identb[:], take(128, 128))
            rte_off = [cur[0] + l * (T * 240) * (2 * NH) for l in range(L)]
            rte_view = [wv(rte_off[l], T * 240, 2 * NH) for l in range(L)]
            ab_sb = wp.tile([128, T], F32, tag="ab")
            nc.sync.dma_start(ab_sb[:], ab_d[:, :])
            abl_sb = wp.tile([128, L * T], F32, tag="abl")
            nc.sync.dma_start(abl_sb[:], abl_d[:, :])

            # ================= adapt (512-wide) =================
            with tc.tile_pool(name="psadapt", bufs=2, space="PSUM") as psa:
                for (t, c0, c1) in sections:
                    for c in range(c0, c1, 512):
                        w = min(512, c1 - c)
                        ft = iop.tile([128, 2, 512], BF16, tag="ft")
                        for k in range(2):
                            nc.sync.dma_start(ft[:, k, 0:w],
                                              featT[k * 128:(k + 1) * 128, c:c + w])
                        ps = psa.tile([128, 512], F32, tag="psadapt")
                        for k in range(2):
                            nc.tensor.matmul(
                                out=ps[:, 0:w],
                                lhsT=aw_sb[:, (t * 2 + k) * NH:(t * 2 + k + 1) * NH],
                                rhs=ft[:, k, 0:w], start=(k == 0), stop=(k == 1))
                        nc.scalar.activation(xT[:, c:c + w], ps[:, 0:w],
                                             mybir.ActivationFunctionType.Tanh,
                                             bias=ab_sb[:, t:t + 1])

            if debug:
                nc.sync.dma_start(xa_out[:, :], xT[:])
            # ================= layers =================
            for l in range(L):
                # ---- tab matmul (owned nodes) ----
                with tc.tile_pool(name="pstab", bufs=2, space="PSUM") as pst:
                    for ci in range(LNP // 128):
                        c = ci * 128
                        t = sec_of[c]
                        ps = pst.tile([128, 3 * NH], F32, tag="pstab")
                        nc.tensor.matmul(
                            out=ps[:], lhsT=xT[:, c:c + 128],
                            rhs=wkvq_sb[:, (l * T + t) * 3 * NH:(l * T + t + 1) * 3 * NH],
                            start=True, stop=False)
                        nc.tensor.matmul(
                            out=ps[:], lhsT=ones1[:],
                            rhs=bkvq_sb[:, (l * T + t) * 3 * NH:(l * T + t + 1) * 3 * NH],
                            start=False, stop=True)
                        tsb = iop.tile([128, 3 * NH], BF16, tag="tsb")
                        nc.vector.tensor_copy(tsb[:], ps[:])
                        nc.sync.dma_start(tab_own[c:c + 128, :], tsb[:])
                # ---- allgather tab ----
                nc.gpsimd.collective_compute(
                    "AllGather", mybir.AluOpType.bypass,
                    replica_groups=[list(range(NCORES))],
                    ins=[tab_own[:, :]], outs=[tab_all[:, :]])
                # ---- zero aggr ----
                nc.sync.dma_start(
                    aggr.rearrange("(a b) c -> a b c", a=128),
                    zero192[:].unsqueeze(1).broadcast_to([128, RT // 128, 192]))
                # ---- edge phase ----
                ctx_eb = tc.tile_pool(name="pseb", bufs=1, space="PSUM")
                ctx_es = tc.tile_pool(name="pses", bufs=1, space="PSUM")
                psb = ctx_eb.__enter__()
                pss = ctx_es.__enter__()
                pos = 0
                for (et, sh, dh, P_g) in groups:
                    if P_g == 0:
                        continue
                    kv_src = tab_all[sh * HALF:(HALF if sh == 0 else RT), 0:2 * NH]
                    q_src = tab_all[dh * HALF:(HALF if dh == 0 else RT),
                                    2 * NH:3 * NH]
                    ag_dst = aggr[dh * HALF:(HALF if dh == 0 else RT), :]
                    for b0 in range(0, P_g, BLK):
                        sl = slice((pos + b0) // 16, (pos + b0 + BLK) // 16)
                        kvt = ep.tile([128, 2, BLK], BF16, tag="kvt")
                        nc.gpsimd.dma_gather(kvt[:], kv_src, kvidx[:, sl], BLK,
                                             BLK, 2 * NH, elem_step=3 * NH,
                                             transpose=True)
                        rtt = ep.tile([128, 2, BLK], BF16, tag="rtt")
                        nc.gpsimd.dma_gather(rtt[:], rte_view[l],
                                             rteidx[:, sl], BLK, BLK, 2 * NH,
                                             transpose=True)
                        qt = ep.tile([128, 1, BLK], BF16, tag="qt")
                        nc.gpsimd.dma_gather(qt[:], q_src, qidx[:, sl], BLK,
                                             BLK, NH, elem_step=3 * NH,
                                             transpose=True)
                        nc.vector.tensor_add(kvt[:], kvt[:], rtt[:])
                        k2 = psb.tile([128, BLK], F32, tag="psk2")
                        nc.tensor.matmul(
                            out=k2[:],
                            lhsT=bdatt_sb[:, (l * R + et) * NH:(l * R + et + 1) * NH],
                            rhs=kvt[:, 0, :], start=True, stop=True)
                        v2 = psb.tile([128, BLK], F32, tag="psv2")
                        nc.tensor.matmul(
                            out=v2[:],
                            lhsT=bdmsg_sb[:, (l * R + et) * NH:(l * R + et + 1) * NH],
                            rhs=kvt[:, 1, :], start=True, stop=True)
                        pT = ep.tile([128, BLK], F32, tag="pT")
                        nc.vector.tensor_tensor(pT[:], qt[:, 0, :], k2[:],
                                                mybir.AluOpType.mult)
                        att = psb.tile([HEADS, BLK], F32, tag="psatt")
                        nc.tensor.matmul(out=att[:], lhsT=headT[:], rhs=pT[:],
                                         start=True, stop=True)
                        ex = ep.tile([HEADS, BLK], F32, tag="ex")
                        nc.scalar.activation(ex[:], att[:],
                                             mybir.ActivationFunctionType.Exp)
                        exb = psb.tile([128, BLK], F32, tag="psexb")
                        nc.tensor.matmul(out=exb[:], lhsT=headb[:], rhs=ex[:],
                                         start=True, stop=True)
                        exbs = ep.tile([128, BLK], BF16, tag="exbs")
                        nc.scalar.activation(exbs[:], exb[:],
                                             mybir.ActivationFunctionType.Copy)
                        msgT = ep.tile([128, BLK], BF16, tag="msgT")
                        nc.vector.tensor_tensor(msgT[:], v2[:], exbs[:],
                                                mybir.AluOpType.mult)
                        exs = ep.tile([HEADS, BLK], BF16, tag="exs")
                        nc.scalar.activation(exs[:], ex[:],
                                             mybir.ActivationFunctionType.Copy)
                        scat = ep.tile([128, BLK // 128, 192], F32, tag="scat")
                        tg0 = (pos + b0) // 128
                        # 4 tiles batched: bank-aligned psum slots avoid any
                        # matmul output crossing a 2KB PSUM bank boundary.
                        idT4 = pss.tile([128, 4, 128], F32, tag="psidT")
                        rows4 = pss.tile([128, 4, 256], BF16, tag="psrows")
                        mg4 = pss.tile([128, 4, 256], F32, tag="psmg")
                        for ti in range(4):
                            nc.tensor.transpose(
                                idT4[:, ti, :],
                                dstid[:, tg0 + ti:tg0 + ti + 1].to_broadcast(
                                    [128, 128]),
                                identf[:])
                            nc.tensor.transpose(
                                rows4[:, ti, 0:128],
                                msgT[:, ti * 128:(ti + 1) * 128], identb[:])
                            nc.tensor.transpose(
                                rows4[:, ti, 128:136],
                                exs[:, ti * 128:(ti + 1) * 128],
                                identb[0:HEADS, 0:HEADS])
                        sel4 = twp.tile([128, 4, 128], BF16, tag="selb")
                        nc.vector.tensor_tensor(
                            sel4[:],
                            dstid[:, tg0:tg0 + 4].unsqueeze(2).broadcast_to(
                                [128, 4, 128]),
                            idT4[:], mybir.AluOpType.is_equal)
                        rowsb = twp.tile([128, 4, 256], BF16, tag="rowsb")
                        nc.scalar.activation(rowsb[:], rows4[:],
                                             mybir.ActivationFunctionType.Copy)
                        for ti in range(4):
                            nc.tensor.matmul(out=mg4[:, ti, 0:136],
                                             lhsT=sel4[:, ti, :],
                                             rhs=rowsb[:, ti, 0:136],
                                             start=True, stop=True)
                        nc.vector.tensor_copy(scat[:, :, 0:136],
                                              mg4[:, :, 0:136])
                        nc.gpsimd.dma_scatter_add(
                            ag_dst, scat[:], scidx[:, sl], BLK, BLK, 192)
                    pos += P_g
                ctx_es.__exit__(None, None, None)
                ctx_eb.__exit__(None, None, None)
                # ---- allreduce aggr ----
                nc.gpsimd.collective_compute(
                    "AllReduce", mybir.AluOpType.add,
                    replica_groups=[list(range(NCORES))],
                    ins=[aggr[:, :]], outs=[aggr2[:, :]])
                if debug and l == 0:
                    nc.sync.dma_start(tab_out[:, :], tab_all[:, :])
                    nc.sync.dma_start(ag_out[:, :], aggr2[:, :])
                # ---- update (512-wide) ----
                with tc.tile_pool(name="psupd", bufs=1, space="PSUM") as psu:
                    for (t, c0, c1) in sections:
                        for c in range(c0, c1, 512):
                            w = min(512, c1 - c)
                            nsub = w // 128
                            asb = iop.tile([128, 4, 192], F32, tag="asb")
                            psM = psu.tile([128, 512], F32, tag="psM")
                            psE = psu.tile([HEADS, 512], F32, tag="psE")
                            for j in range(nsub):
                                ci = (c + j * 128) // 128
                                nc.gpsimd.indirect_dma_start(
                                    out=asb[:, j, :], out_offset=None,
                                    in_=aggr2[:, :],
                                    in_offset=bass.IndirectOffsetOnAxis(
                                        ap=updidx[:, ci:ci + 1], axis=0))
                                nc.tensor.transpose(
                                    psM[:, j * 128:(j + 1) * 128],
                                    asb[:, j, 0:128], identf[:])
                                nc.tensor.transpose(
                                    psE[:, j * 128:(j + 1) * 128],
                                    asb[:, j, 128:136], identf[:])
                            den = twp.tile([HEADS, 512], F32, tag="den")
                            nc.vector.tensor_scalar_add(den[:, 0:w], psE[:, 0:w],
                                                        1e-16)
                            rec = twp.tile([HEADS, 512], F32, tag="rec")
                            nc.vector.reciprocal(rec[:, 0:w], den[:, 0:w])
                            psD = psu.tile([128, 512], F32, tag="psD")
                            nc.tensor.matmul(out=psD[:, 0:w], lhsT=headb[:],
                                             rhs=rec[:, 0:w], start=True,
                                             stop=True)
                            dens = twp.tile([128, 512], F32, tag="dens")
                            nc.vector.tensor_copy(dens[:, 0:w], psD[:, 0:w])
                            hsb = twp.tile([128, 512], F32, tag="hsb")
                            nc.vector.tensor_tensor(hsb[:, 0:w], psM[:, 0:w],
                                                    dens[:, 0:w],
                                                    mybir.AluOpType.mult)
                            hgl = twp.tile([128, 512], BF16, tag="hgl")
                            nc.scalar.activation(hgl[:, 0:w], hsb[:, 0:w],
                                                 mybir.ActivationFunctionType.Gelu)
                            psT = psu.tile([128, 512], F32, tag="psT")
                            nc.tensor.matmul(
                                out=psT[:, 0:w],
                                lhsT=awl_sb[:, (l * T + t) * NH:(l * T + t + 1) * NH],
                                rhs=hgl[:, 0:w], start=True, stop=True)
                            tmp = twp.tile([128, 512], F32, tag="tmp")
                            nc.scalar.activation(
                                tmp[:, 0:w], psT[:, 0:w],
                                mybir.ActivationFunctionType.Identity,
                                bias=abl_sb[:, l * T + t:l * T + t + 1])
                            alpha = ALPHAS[l][t]
                            nc.vector.scalar_tensor_tensor(
                                xT[:, c:c + w], xT[:, c:c + w], 1.0 - alpha,
                                tmp[:, 0:w], mybir.AluOpType.mult,
                                mybir.AluOpType.add)

            # ================= output =================
            nc.sync.dma_start(xout[:, :], xT[:])
    nc.compile()
    return nc


ALPHAS = None  # set by kernel() before build (sigmoid(skip), [L][T])


_init_th = None


def _warmup_build():
    """Trigger bass/tile lazy init (cffi cdef parse, isa tables) with a tiny
    throwaway build so the real build doesn't pay the ~0.6s warmup."""
    try:
        nc = bacc.Bacc("TRN2", target_bir_lowering=False, debug=False,
                       num_devices=NCORES)
        a = nc.dram_tensor("a", [128, 128], BF16, kind="ExternalInput").ap()
        o = nc.dram_tensor("o", [128, 128], F32, kind="ExternalOutput").ap()
        with tile.TileContext(nc) as tc:
            with tc.tile_pool(name="w", bufs=1) as w, \
                 tc.tile_pool(name="p", bufs=1, space="PSUM") as p:
                t = w.tile([128, 128], BF16, tag="t")
                nc.sync.dma_start(t[:], a[:, :])
                ps = p.tile([128, 128], F32, tag="ps")
                nc.tensor.matmul(out=ps[:], lhsT=t[:], rhs=t[:], start=True,
                                 stop=True)
                t2 = w.tile([128, 128], F32, tag="t2")
                nc.vector.tensor_copy(t2[:], ps[:])
                nc.sync.dma_start(o[:, :], t2[:])
        nc.compile()
    except Exception:
        pass


def _start_jax_init():
    """Kick off jax/axon backend init + bass warmup in the background."""
    global _init_th
    if _init_th is None:
        import threading

        def work():
            import jax
            jax.devices()

        _init_th = threading.Thread(target=work)
        _init_th.start()
        threading.Thread(target=_warmup_build).start()


def _prefetch(make_in_maps, out_shapes):
    """Build per-core input arrays and device_put them over the mesh in a
    background thread, overlapping packing + tunnel transfer with NEFF
    build."""
    import threading
    placed = {}

    def work():
        if _init_th is not None:
            _init_th.join()
        import jax
        from jax.sharding import Mesh, PartitionSpec, NamedSharding
        devices = jax.devices()[:NCORES]
        mesh = Mesh(np.asarray(devices), ("core",))
        shd = NamedSharding(mesh, PartitionSpec("core"))
        in_maps = make_in_maps()
        for name in in_maps[0]:
            cat = np.concatenate([np.asarray(in_maps[c][name])
                                  for c in range(NCORES)], axis=0)
            placed[name] = jax.device_put(cat, shd)
        import jax.numpy as jnp
        for name, (shape, dtype) in out_shapes.items():
            full = (NCORES * shape[0], *shape[1:])
            placed['__zero__' + name] = jax.jit(
                lambda full=full, dtype=dtype: jnp.zeros(full, dtype),
                out_shardings=shd)()

    th = threading.Thread(target=work)
    th.start()
    return placed, th


def _run_pre(nc, placed, th):
    """run_bass_via_pjrt (multi-core axon branch) with pre-placed inputs."""
    import jax
    from jax.experimental.shard_map import shard_map
    from jax.sharding import Mesh, PartitionSpec
    from concourse import bass2jax
    bass2jax.install_neuronx_cc_hook()
    assert nc.dbg_addr is None
    partition_name = (nc.partition_id_tensor.name
                      if nc.partition_id_tensor else None)
    in_names, out_names, out_avals = [], [], []
    for alloc in nc.m.functions[0].allocations:
        if not isinstance(alloc, mybir.MemoryLocationSet):
            continue
        name = alloc.memorylocations[0].name
        if alloc.kind == "ExternalInput":
            if name != partition_name:
                in_names.append(name)
        elif alloc.kind == "ExternalOutput":
            assert alloc.tensor_shape is not None and alloc.dtype is not None
            out_names.append(name)
            out_avals.append(jax.core.ShapedArray(
                tuple(alloc.tensor_shape), mybir.dt.np(alloc.dtype)))
    n_params = len(in_names)
    all_names = in_names + out_names
    if partition_name is not None:
        all_names.append(partition_name)
    donate = tuple(range(n_params, n_params + len(out_names)))

    def _body(*args):
        operands = list(args)
        if partition_name is not None:
            operands.append(bass2jax.partition_id_tensor())
        outs = bass2jax._bass_exec_p.bind(
            *operands, out_avals=tuple(out_avals), in_names=tuple(all_names),
            out_names=tuple(out_names), lowering_input_output_aliases=(),
            sim_require_finite=True, sim_require_nnan=True, nc=nc)
        return tuple(outs)

    devices = jax.devices()[:NCORES]
    mesh = Mesh(np.asarray(devices), ("core",))
    P = PartitionSpec
    in_specs = (P("core"),) * (n_params + len(out_names))
    out_specs = (P("core"),) * len(out_names)
    sharded = jax.jit(
        shard_map(_body, mesh=mesh, in_specs=in_specs, out_specs=out_specs,
                  check_rep=False),
        donate_argnums=donate, keep_unused=True)
    _tick('join upload')
    th.join()
    _tick('exec')
    ins = [placed[name] for name in in_names]
    zeros = [placed['__zero__' + name] for name in out_names]
    out_arrs = sharded(*ins, *zeros)
    for a in out_arrs:
        a.block_until_ready()
    _tick('download')
    out = {
        name: np.asarray(out_arrs[i]).reshape(NCORES, *out_avals[i].shape)
        for i, name in enumerate(out_names)
    }
    _tick('download done')
    return out


def wrap16(a):
    return np.ascontiguousarray(a.reshape(-1, 16).T)


def _bf(x):
    return np.ascontiguousarray(x).astype(ml_dtypes.bfloat16)


def kernel(node_feature, adapt_w, adapt_b, k_w, k_b, q_w, q_b, v_w, v_b,
           a_w, a_b, rel_pri, rel_att, rel_msg, skip, rte_tab, rte_w, rte_b,
           node_type, edge_index, edge_type, edge_time):
    global ALPHAS
    _tick('kernel start')
    _start_jax_init()
    node_type = np.asarray(node_type).astype(np.int64)
    src = np.asarray(edge_index[0]).astype(np.int64)
    dst = np.asarray(edge_index[1]).astype(np.int64)
    et = np.asarray(edge_type).astype(np.int64)
    etime = np.asarray(edge_time).astype(np.int64)

    # ---- node partitioning ----
    order = np.argsort(node_type, kind='stable')
    own = [order[c::NCORES] for c in range(NCORES)]
    cnt = np.stack([np.bincount(node_type[o], minlength=T) for o in own])
    tpad = [int(np.ceil((cnt[:, t].max() + 1) / 128) * 128) for t in range(T)]
    LNP = int(sum(tpad))
    offs = np.cumsum([0] + tpad)[:-1]
    sections = [(t, int(offs[t]), int(offs[t] + tpad[t])) for t in range(T)]
    loc2glob = np.full((NCORES, LNP), -1, np.int64)
    for c in range(NCORES):
        o = own[c]
        for t in range(T):
            sec = o[node_type[o] == t]
            loc2glob[c, offs[t]:offs[t] + len(sec)] = sec
    valid = loc2glob >= 0
    l2g0 = np.where(valid, loc2glob, 0)
    row_of = np.empty(N, np.int64)
    for c in range(NCORES):
        row_of[loc2glob[c][valid[c]]] = c * LNP + np.flatnonzero(valid[c])
    RT = NCORES * LNP
    dummy_lo = int(offs[0] + cnt[0, 0])                      # core0 pad row
    dummy_hi = int(7 * LNP + offs[0] + cnt[7, 0])            # core7 pad row
    assert dummy_lo < HALF and HALF <= dummy_hi < RT

    # ---- edge partitioning: 16 groups x 8 cores, dst-sorted, packed ----
    _tick('edge prep start')
    srow = row_of[src]
    drow = row_of[dst]
    st = node_type[src]
    sh = (srow >= HALF).astype(np.int64)
    dh = (drow >= HALF).astype(np.int64)
    garr = et * 4 + sh * 2 + dh

    # per (group, core): edge id lists (dst-sorted), packed into BLK blocks
    # with no dst run straddling a block boundary.
    P_gs = []
    core_slots = [[] for _ in range(NCORES)]   # per core: list of arrays (edge id or -1 pad)
    for g in range(16):
        eg = np.flatnonzero(garr == g)
        eg = eg[np.argsort(drow[eg], kind='stable')]
        nb = len(eg)
        bounds = [nb * c // NCORES for c in range(NCORES + 1)]
        packed = []
        for c in range(NCORES):
            ch = eg[bounds[c]:bounds[c + 1]]
            if len(ch) == 0:
                packed.append(np.zeros(0, np.int64))
                continue
            d = drow[ch]
            runstart = np.flatnonzero(np.r_[True, np.diff(d) != 0])
            runlen = np.diff(np.r_[runstart, len(ch)])
            assert runlen.max() <= 128
            slots = []
            posn = 0
            for rs, rl in zip(runstart, runlen):
                # no dst run may straddle a 128-tile boundary: two leader
                # tokens for one row in a scatter call would race (CCE add)
                rem = 128 - posn % 128
                if rl > rem and posn % 128 != 0:
                    slots.append(np.full(rem, -1, np.int64))
                    posn += rem
                slots.append(ch[rs:rs + rl])
                posn += rl
            arr = np.concatenate(slots)
            packed.append(arr)
        mx = max(len(p) for p in packed)
        P_g = int(np.ceil(mx / BLK) * BLK) if mx else 0
        P_gs.append(P_g)
        for c in range(NCORES):
            p = packed[c]
            core_slots[c].append(np.r_[p, np.full(P_g - len(p), -1, np.int64)])
    EP = int(sum(P_gs))
    NT = EP // 128
    NCH = LNP // 128

    # per-core index arrays
    kvidx_m, rteidx_m, qidx_m, scidx_m, dstid_m = [], [], [], [], []
    for c in range(NCORES):
        eids = np.concatenate(core_slots[c])          # [EP], -1 = pad
        pad = eids < 0
        e0 = np.where(pad, 0, eids)
        kvi = (srow[e0] - HALF * sh[e0]).astype(np.int64)
        rti = (st[e0] * 240 + etime[e0]).astype(np.int64)
        qi = (drow[e0] - HALF * dh[e0]).astype(np.int64)
        kvi[pad] = 0; rti[pad] = 0; qi[pad] = 0
        did = drow[e0].astype(np.float64)
        did[pad] = -1.0
        # leaders: first slot of its dst within each 128-tile
        dd = drow[e0]; dd[pad] = -1
        lead = np.r_[True, dd[1:] != dd[:-1]]
        lead[::128] = True
        lead &= ~pad
        # scatter idx: leader -> real (half-relative), else dummy of the half
        pos = 0
        si = np.empty(EP, np.int64)
        gi = 0
        for (g, P_g) in enumerate(P_gs):
            dhh = g & 1
            dmy = dummy_lo if dhh == 0 else dummy_hi - HALF
            seg = slice(pos, pos + P_g)
            si[seg] = np.where(lead[seg], qi[seg], dmy)
            pos += P_g
        kvidx_m.append(wrap16(kvi.astype(np.int16)))
        rteidx_m.append(wrap16(rti.astype(np.int16)))
        qidx_m.append(wrap16(qi.astype(np.int16)))
        scidx_m.append(wrap16(si.astype(np.int16)))
        dv = np.zeros((128, NT), np.float32)
        dv[np.arange(EP) % 128, np.arange(EP) // 128] = did
        dstid_m.append(dv)
    _tick('edge prep done')

    # ---- weights folding ----
    pri = np.asarray(rel_pri, np.float32)
    ALPHAS = [[float(1.0 / (1.0 + np.exp(-np.asarray(skip, np.float32)[l, t])))
               for t in range(T)] for l in range(L)]

    def bd(mats):  # [H,DK,DK] -> block-diag [NH,NH]
        out = np.zeros((NH, NH), np.float32)
        for h in range(HEADS):
            out[h * DK:(h + 1) * DK, h * DK:(h + 1) * DK] = mats[h]
        return out

    bdatt = np.zeros((L, R, NH, NH), np.float32)
    bdmsg = np.zeros((L, R, NH, NH), np.float32)
    for l in range(L):
        for r in range(R):
            bdatt[l, r] = bd(np.asarray(rel_att[l, r], np.float32)
                             * (pri[l, r][:, None, None] / SQRT_DK))
            bdmsg[l, r] = bd(np.asarray(rel_msg[l, r], np.float32))
    wkvq = np.zeros((L, T, NH, 3 * NH), np.float32)
    bkvq = np.zeros((L * T, 3 * NH), np.float32)
    rte_kv = np.zeros((L, T * 240, 2 * NH), np.float32)
    awl = np.zeros((L, T, NH, NH), np.float32)
    abl = np.zeros((NH, L * T), np.float32)
    for l in range(L):
        kw = np.asarray(k_w[l], np.float32); kb = np.asarray(k_b[l], np.float32)
        qw = np.asarray(q_w[l], np.float32); qb = np.asarray(q_b[l], np.float32)
        vw = np.asarray(v_w[l], np.float32); vb = np.asarray(v_b[l], np.float32)
        rte = (np.asarray(rte_tab[l], np.float32) @ np.asarray(rte_w[l], np.float32)
               + np.asarray(rte_b[l], np.float32))     # [240, NH]
        for t in range(T):
            wkvq[l, t, :, 0:NH] = kw[t]
            wkvq[l, t, :, NH:2 * NH] = vw[t]
            wkvq[l, t, :, 2 * NH:] = qw[t]
            bkvq[l * T + t, 0:NH] = kb[t]
            bkvq[l * T + t, NH:2 * NH] = vb[t]
            bkvq[l * T + t, 2 * NH:] = qb[t]
            rte_kv[l, t * 240:(t + 1) * 240, 0:NH] = rte @ kw[t]
            rte_kv[l, t * 240:(t + 1) * 240, NH:] = rte @ vw[t]
        for t in range(T):
            awl[l, t] = np.asarray(a_w[l, t], np.float32) * ALPHAS[l][t]
            abl[:, l * T + t] = np.asarray(a_b[l, t], np.float32) * ALPHAS[l][t]

    headT = np.zeros((NH, HEADS), np.float32)
    headb = np.zeros((HEADS, NH), np.float32)
    for h in range(HEADS):
        headT[h * DK:(h + 1) * DK, h] = 1.0
        headb[h, h * DK:(h + 1) * DK] = 1.0

    # ---- pack bf16 weights into one blob (order must match device take()) ----
    aw_f = np.asarray(adapt_w, np.float32)
    parts = []
    for t in range(T):
        for k in range(2):
            parts.append(aw_f[t, k * 128:(k + 1) * 128, :])
    for l in range(L):
        for t in range(T):
            parts.append(wkvq[l, t])
    for i in range(L * T):
        parts.append(bkvq[i])
    for l in range(L):
        for r in range(R):
            parts.append(bdatt[l, r])
            parts.append(bdmsg[l, r])
    for l in range(L):
        for t in range(T):
            parts.append(awl[l, t])
    parts.append(np.eye(128, dtype=np.float32))
    parts.append(rte_kv.reshape(-1, 2 * NH))
    blob = np.concatenate([np.ascontiguousarray(p, np.float32).ravel()
                           for p in parts]).astype(ml_dtypes.bfloat16)
    WSHARD = int(np.ceil(len(blob) / (NCORES * 64)) * 64)
    blob = np.r_[blob, np.zeros(NCORES * WSHARD - len(blob),
                                ml_dtypes.bfloat16)]

    # ---- in_maps (built inside the prefetch thread) ----
    nf = np.asarray(node_feature, np.float32)

    def make_in_maps():
        ab_host = np.asarray(adapt_b, np.float32).T.copy()      # [NH, T]
        shared = {
            "ab": ab_host, "abl": abl,
            "headT": headT, "headb": headb,
            "ones1": np.ones((1, NH), ml_dtypes.bfloat16),
            "identf": np.eye(128, dtype=np.float32),
        }
        in_maps = []
        for c in range(NCORES):
            featT = nf[l2g0[c]].T.copy()
            featT[:, ~valid[c]] = 0
            upd = (c * LNP + np.arange(LNP)).reshape(NCH, 128).T.astype(np.int32)
            m = dict(shared)
            m.update({
                "featT": _bf(featT),
                "wblob": blob[c * WSHARD:(c + 1) * WSHARD],
                "kvidx": kvidx_m[c], "rteidx": rteidx_m[c],
                "qidx": qidx_m[c], "scidx": scidx_m[c],
                "dstid": dstid_m[c], "updidx": np.ascontiguousarray(upd),
            })
            in_maps.append(m)
        return in_maps

    _tick('prefetch start')
    placed, th = _prefetch(
        make_in_maps, {"xout": ((NH, LNP), ml_dtypes.bfloat16)})

    # ---- build / compile (overlaps with upload thread) ----
    key = (LNP, tuple(P_gs), WSHARD)
    if key not in _cache:
        _tick('build+compile start')
        _cache[key] = build_neff(LNP, sections, P_gs, NT, NCH, WSHARD)
        _tick('build+compile done')
    nc = _cache[key]

    _tick('launch')
    outs = _run_pre(nc, placed, th)
    _tick('launch done')
    x = np.zeros((N, NH), np.float32)
    for c in range(NCORES):
        xo = outs["xout"][c].astype(np.float32)                  # [NH, LNP]
        x[loc2glob[c][valid[c]]] = xo.T[valid[c]]
    _tick('done')
    return x


# revision 38
# speedup vs baseline: 1.1466x; 1.1466x over previous
"""HGT (heterogeneous graph transformer) on 8 TRN2 NeuronCores.

Single-launch, fully on-device design:
  - Nodes type-sorted, dealt round-robin to 8 cores (SPMD: one NEFF).
  - Device: adapt (tanh per-type linear) -> per layer: per-type K|V|Q table
    matmul for owned nodes -> AllGather table (bf16) -> edge phase on device
    (dma_gather of K/V rows by src, RTE rows by (type,time), Q rows by dst;
    per-relation block-diag matmuls; unstabilized segment softmax via exp +
    scatter-added denominators; selection-matrix merge per 128-edge tile;
    dma_scatter_add into aggr table) -> AllReduce aggr -> normalize / gelu /
    per-type update matmul -> residual mix, all in SBUF-resident xT.
  - Edges sorted by (edge_type, src_half, dst_half, dst_row): 16 groups.
    Halves solve the int16 gather-index range; relation grouping gives
    sliced block-diag matmuls; dst sort + per-tile leader tokens give
    collision-free scatter-adds.
  - Host: only integer index prep + bf16 packing + final transpose.
"""
import sys, math, os, time
sys.path.insert(0, '/opt/trn_rl_repo')
import numpy as np
import ml_dtypes

import concourse.bass as bass
import concourse.tile as tile
from concourse import bacc, mybir
from concourse import bass_utils

BF16 = mybir.dt.bfloat16
F32 = mybir.dt.float32
I16 = mybir.dt.int16
I32 = mybir.dt.int32
NCORES = 8
N, E = 50000, 300000
IN_DIM, NH, HEADS, DK = 256, 128, 8, 16
T, R, L = 3, 4, 2
SQRT_DK = math.sqrt(DK)
HALF = 32768
BLK = 512

_T0 = time.time()
def _tick(msg):
    print(f"[kernel +{time.time()-_T0:7.2f}s] {msg}", file=sys.stderr, flush=True)

_cache = {}


def build_neff(LNP, sections, P_gs, NT, NCH, WSHARD, debug=False):
    """One SPMD NEFF for the whole model.

    sections: list of (type, c0, c1) node ranges (multiples of 128).
    P_gs: list of 16 per-group padded edge counts (multiples of BLK).
    NT = sum(P_gs)//128 tile count; NCH = LNP//128 node chunk count.
    """
    RT = NCORES * LNP
    EP = sum(P_gs)
    nc = bacc.Bacc("TRN2", target_bir_lowering=False, debug=False,
                   num_devices=NCORES)
    # ---- inputs ----
    featT = nc.dram_tensor("featT", [IN_DIM, LNP], BF16, kind="ExternalInput").ap()
    wsh_d = nc.dram_tensor("wblob", [WSHARD], BF16, kind="ExternalInput").ap()
    ab_d = nc.dram_tensor("ab", [NH, T], F32, kind="ExternalInput").ap()
    abl_d = nc.dram_tensor("abl", [NH, L * T], F32, kind="ExternalInput").ap()
    headT_d = nc.dram_tensor("headT", [NH, HEADS], F32, kind="ExternalInput").ap()
    headb_d = nc.dram_tensor("headb", [HEADS, NH], F32, kind="ExternalInput").ap()
    ones1_d = nc.dram_tensor("ones1", [1, NH], BF16, kind="ExternalInput").ap()
    identf_d = nc.dram_tensor("identf", [128, 128], F32, kind="ExternalInput").ap()
    wblob = nc.dram_tensor("wblob_all", [NCORES * WSHARD], BF16,
                           kind="Internal", addr_space="Shared").ap()
    wstage = nc.dram_tensor("wstage", [WSHARD], BF16, kind="Internal").ap()

    def wv(off, rows, cols):
        return wblob[off:off + rows * cols].rearrange("(a b) -> a b", a=rows)
    kvidx_d = nc.dram_tensor("kvidx", [16, EP // 16], I16, kind="ExternalInput").ap()
    rteidx_d = nc.dram_tensor("rteidx", [16, EP // 16], I16, kind="ExternalInput").ap()
    qidx_d = nc.dram_tensor("qidx", [16, EP // 16], I16, kind="ExternalInput").ap()
    scidx_d = nc.dram_tensor("scidx", [16, EP // 16], I16, kind="ExternalInput").ap()
    dstid_d = nc.dram_tensor("dstid", [128, NT], F32, kind="ExternalInput").ap()
    updidx_d = nc.dram_tensor("updidx", [128, NCH], I32, kind="ExternalInput").ap()
    # ---- output ----
    xout = nc.dram_tensor("xout", [NH, LNP], BF16, kind="ExternalOutput").ap()
    if debug:
        xa_out = nc.dram_tensor("xa_out", [NH, LNP], BF16, kind="ExternalOutput").ap()
        tab_out = nc.dram_tensor("tab_out", [NCORES * LNP, 3 * NH], BF16, kind="ExternalOutput").ap()
        ag_out = nc.dram_tensor("ag_out", [NCORES * LNP, 192], F32, kind="ExternalOutput").ap()
    # ---- internal DRAM ----
    tab_own = nc.dram_tensor("tab_own", [LNP, 3 * NH], BF16, kind="Internal").ap()
    tab_all = nc.dram_tensor("tab_all", [RT, 3 * NH], BF16, kind="Internal",
                             addr_space="Shared").ap()
    aggr = nc.dram_tensor("aggr", [RT, 192], F32, kind="Internal").ap()
    aggr2 = nc.dram_tensor("aggr2", [RT, 192], F32, kind="Internal",
                           addr_space="Shared").ap()

    sec_of = {}
    for (t, c0, c1) in sections:
        for c in range(c0, c1, 128):
            sec_of[c] = t

    groups = []  # (et, sh, dh, P_g)
    gi = 0
    for et in range(R):
        for sh in range(2):
            for dh in range(2):
                groups.append((et, sh, dh, P_gs[gi]))
                gi += 1

    with tile.TileContext(nc) as tc:
        with tc.tile_pool(name="persist", bufs=1) as pp, \
             tc.tile_pool(name="wts", bufs=1) as wp, \
             tc.tile_pool(name="io", bufs=3) as iop, \
             tc.tile_pool(name="edge", bufs=2) as ep, \
             tc.tile_pool(name="tilework", bufs=2) as twp:
            # ================= persistent loads =================
            nc.sync.dma_start(wstage[:], wsh_d[:])
            nc.gpsimd.collective_compute(
                "AllGather", mybir.AluOpType.bypass,
                replica_groups=[list(range(NCORES))],
                ins=[wstage[:]], outs=[wblob[:]])
            xT = pp.tile([128, LNP], BF16, tag="xT")
            kvidx = pp.tile([128, EP // 16], I16, tag="kvidx")
            rteidx = pp.tile([128, EP // 16], I16, tag="rteidx")
            qidx = pp.tile([128, EP // 16], I16, tag="qidx")
            scidx = pp.tile([128, EP // 16], I16, tag="scidx")
            for (sb, d) in ((kvidx, kvidx_d), (rteidx, rteidx_d),
                            (qidx, qidx_d), (scidx, scidx_d)):
                for k in range(8):
                    nc.sync.dma_start(sb[16 * k:16 * (k + 1), :], d[:, :])
            dstid = pp.tile([128, NT], F32, tag="dstid")
            nc.sync.dma_start(dstid[:], dstid_d[:, :])
            updidx = pp.tile([128, NCH], I32, tag="updidx")
            nc.sync.dma_start(updidx[:], updidx_d[:, :])
            headT = pp.tile([128, HEADS], F32, tag="headT")
            nc.sync.dma_start(headT[:], headT_d[:, :])
            headb = pp.tile([HEADS, 128], F32, tag="headb")
            nc.sync.dma_start(headb[:], headb_d[:, :])
            ones1 = pp.tile([1, 128], BF16, tag="ones1")
            nc.sync.dma_start(ones1[:], ones1_d[:, :])
            identf = pp.tile([128, 128], F32, tag="identf")
            nc.sync.dma_start(identf[:], identf_d[:, :])
            zero192 = pp.tile([128, 192], F32, tag="zero192")
            nc.vector.memset(zero192[:], 0.0)
            # weights (from allgathered blob; offsets match host pack order)
            cur = [0]

            def take(rows, cols):
                v = wv(cur[0], rows, cols)
                cur[0] += rows * cols
                return v

            aw_sb = wp.tile([128, T * 2 * NH], BF16, tag="aw")
            for t in range(T):
                for k in range(2):
                    nc.sync.dma_start(
                        aw_sb[:, (t * 2 + k) * NH:(t * 2 + k + 1) * NH],
                        take(128, NH))
            wkvq_sb = wp.tile([128, L * T * 3 * NH], BF16, tag="wkvq")
            for l in range(L):
                for t in range(T):
                    nc.sync.dma_start(
                        wkvq_sb[:, (l * T + t) * 3 * NH:(l * T + t + 1) * 3 * NH],
                        take(NH, 3 * NH))
            bkvq_sb = wp.tile([1, L * T * 3 * NH], BF16, tag="bkvq")
            for i in range(L * T):
                nc.sync.dma_start(bkvq_sb[:, i * 3 * NH:(i + 1) * 3 * NH],
                                  take(1, 3 * NH))
            bdatt_sb = wp.tile([128, L * R * NH], BF16, tag="bdatt")
            bdmsg_sb = wp.tile([128, L * R * NH], BF16, tag="bdmsg")
            for l in range(L):
                for r in range(R):
                    nc.sync.dma_start(
                        bdatt_sb[:, (l * R + r) * NH:(l * R + r + 1) * NH],
                        take(NH, NH))
                    nc.sync.dma_start(
                        bdmsg_sb[:, (l * R + r) * NH:(l * R + r + 1) * NH],
                        take(NH, NH))
            awl_sb = wp.tile([128, L * T * NH], BF16, tag="awl")
            for l in range(L):
                for t in range(T):
                    nc.sync.dma_start(
                        awl_sb[:, (l * T + t) * NH:(l * T + t + 1) * NH],
                        take(NH, NH))
            identb = pp.tile([128, 128], BF16, tag="identb")
            nc.sync.dma_start(identb[:], take(128, 128))
            rte_off = [cur[0] + l * (T * 240) * (2 * NH) for l in range(L)]
            rte_view = [wv(rte_off[l], T * 240, 2 * NH) for l in range(L)]
            ab_sb = wp.tile([128, T], F32, tag="ab")
            nc.sync.dma_start(ab_sb[:], ab_d[:, :])
            abl_sb = wp.tile([128, L * T], F32, tag="abl")
            nc.sync.dma_start(abl_sb[:], abl_d[:, :])

            # ================= adapt (512-wide) =================
            with tc.tile_pool(name="psadapt", bufs=2, space="PSUM") as psa:
                for (t, c0, c1) in sections:
                    for c in range(c0, c1, 512):
                        w = min(512, c1 - c)
                        ft = iop.tile([128, 2, 512], BF16, tag="ft")
                        for k in range(2):
                            nc.sync.dma_start(ft[:, k, 0:w],
                                              featT[k * 128:(k + 1) * 128, c:c + w])
                        ps = psa.tile([128, 512], F32, tag="psadapt")
                        for k in range(2):
                            nc.tensor.matmul(
                                out=ps[:, 0:w],
                                lhsT=aw_sb[:, (t * 2 + k) * NH:(t * 2 + k + 1) * NH],
                                rhs=ft[:, k, 0:w], start=(k == 0), stop=(k == 1))
                        nc.scalar.activation(xT[:, c:c + w], ps[:, 0:w],
                                             mybir.ActivationFunctionType.Tanh,
                                             bias=ab_sb[:, t:t + 1])

            if debug:
                nc.sync.dma_start(xa_out[:, :], xT[:])
            # ================= layers =================
            for l in range(L):
                # ---- tab matmul (owned nodes) ----
                with tc.tile_pool(name="pstab", bufs=2, space="PSUM") as pst:
                    for ci in range(LNP // 128):
                        c = ci * 128
                        t = sec_of[c]
                        ps = pst.tile([128, 3 * NH], F32, tag="pstab")
                        nc.tensor.matmul(
                            out=ps[:], lhsT=xT[:, c:c + 128],
                            rhs=wkvq_sb[:, (l * T + t) * 3 * NH:(l * T + t + 1) * 3 * NH],
                            start=True, stop=False)
                        nc.tensor.matmul(
                            out=ps[:], lhsT=ones1[:],
                            rhs=bkvq_sb[:, (l * T + t) * 3 * NH:(l * T + t + 1) * 3 * NH],
                            start=False, stop=True)
                        tsb = iop.tile([128, 3 * NH], BF16, tag="tsb")
                        nc.vector.tensor_copy(tsb[:], ps[:])
                        nc.sync.dma_start(tab_own[c:c + 128, :], tsb[:])
                # ---- allgather tab ----
                nc.gpsimd.collective_compute(
                    "AllGather", mybir.AluOpType.bypass,
                    replica_groups=[list(range(NCORES))],
                    ins=[tab_own[:, :]], outs=[tab_all[:, :]])
                # ---- zero aggr ----
                nc.sync.dma_start(
                    aggr.rearrange("(a b) c -> a b c", a=128),
                    zero192[:].unsqueeze(1).broadcast_to([128, RT // 128, 192]))
                # ---- edge phase ----
                ctx_eb = tc.tile_pool(name="pseb", bufs=1, space="PSUM")
                ctx_es = tc.tile_pool(name="pses", bufs=1, space="PSUM")
                psb = ctx_eb.__enter__()
                pss = ctx_es.__enter__()
                pos = 0
                for (et, sh, dh, P_g) in groups:
                    if P_g == 0:
                        continue
                    kv_src = tab_all[sh * HALF:(HALF if sh == 0 else RT), 0:2 * NH]
                    q_src = tab_all[dh * HALF:(HALF if dh == 0 else RT),
                                    2 * NH:3 * NH]
                    ag_dst = aggr[dh * HALF:(HALF if dh == 0 else RT), :]
                    for b0 in range(0, P_g, BLK):
                        sl = slice((pos + b0) // 16, (pos + b0 + BLK) // 16)
                        kvt = ep.tile([128, 2, BLK], BF16, tag="kvt")
                        nc.gpsimd.dma_gather(kvt[:], kv_src, kvidx[:, sl], BLK,
                                             BLK, 2 * NH, elem_step=3 * NH,
                                             transpose=True)
                        rtt = ep.tile([128, 2, BLK], BF16, tag="rtt")
                        nc.gpsimd.dma_gather(rtt[:], rte_view[l],
                                             rteidx[:, sl], BLK, BLK, 2 * NH,
                                             transpose=True)
                        qt = ep.tile([128, 1, BLK], BF16, tag="qt")
                        nc.gpsimd.dma_gather(qt[:], q_src, qidx[:, sl], BLK,
                                             BLK, NH, elem_step=3 * NH,
                                             transpose=True)
                        nc.vector.tensor_add(kvt[:], kvt[:], rtt[:])
                        k2 = psb.tile([128, BLK], F32, tag="psk2")
                        nc.tensor.matmul(
                            out=k2[:],
                            lhsT=bdatt_sb[:, (l * R + et) * NH:(l * R + et + 1) * NH],
                            rhs=kvt[:, 0, :], start=True, stop=True)
                        v2 = psb.tile([128, BLK], F32, tag="psv2")
                        nc.tensor.matmul(
                            out=v2[:],
                            lhsT=bdmsg_sb[:, (l * R + et) * NH:(l * R + et + 1) * NH],
                            rhs=kvt[:, 1, :], start=True, stop=True)
                        pT = ep.tile([128, BLK], F32, tag="pT")
                        nc.vector.tensor_tensor(pT[:], qt[:, 0, :], k2[:],
                                                mybir.AluOpType.mult)
                        att = psb.tile([HEADS, BLK], F32, tag="psatt")
                        nc.tensor.matmul(out=att[:], lhsT=headT[:], rhs=pT[:],
                                         start=True, stop=True)
                        ex = ep.tile([HEADS, BLK], F32, tag="ex")
                        nc.scalar.activation(ex[:], att[:],
                                             mybir.ActivationFunctionType.Exp)
                        exb = psb.tile([128, BLK], F32, tag="psexb")
                        nc.tensor.matmul(out=exb[:], lhsT=headb[:], rhs=ex[:],
                                         start=True, stop=True)
                        exbs = ep.tile([128, BLK], BF16, tag="exbs")
                        nc.scalar.activation(exbs[:], exb[:],
                                             mybir.ActivationFunctionType.Copy)
                        msgT = ep.tile([128, BLK], BF16, tag="msgT")
                        nc.vector.tensor_tensor(msgT[:], v2[:], exbs[:],
                                                mybir.AluOpType.mult)
                        exs = ep.tile([HEADS, BLK], BF16, tag="exs")
                        nc.scalar.activation(exs[:], ex[:],
                                             mybir.ActivationFunctionType.Copy)
                        scat = ep.tile([128, BLK // 128, 192], F32, tag="scat")
                        tg0 = (pos + b0) // 128
                        # 4 tiles batched: bank-aligned psum slots avoid any
                        # matmul output crossing a 2KB PSUM bank boundary.
                        idT4 = pss.tile([128, 4, 128], F32, tag="psidT")
                        rows4 = pss.tile([128, 4, 256], BF16, tag="psrows")
                        mg4 = pss.tile([128, 4, 256], F32, tag="psmg")
                        for ti in range(4):
                            nc.tensor.transpose(
                                idT4[:, ti, :],
                                dstid[:, tg0 + ti:tg0 + ti + 1].to_broadcast(
                                    [128, 128]),
                                identf[:])
                            nc.tensor.transpose(
                                rows4[:, ti, 0:128],
                                msgT[:, ti * 128:(ti + 1) * 128], identb[:])
                            nc.tensor.transpose(
                                rows4[:, ti, 128:136],
                                exs[:, ti * 128:(ti + 1) * 128],
                                identb[0:HEADS, 0:HEADS])
                        sel4 = twp.tile([128, 4, 128], BF16, tag="selb")
                        nc.vector.tensor_tensor(
                            sel4[:],
                            dstid[:, tg0:tg0 + 4].unsqueeze(2).broadcast_to(
                                [128, 4, 128]),
                            idT4[:], mybir.AluOpType.is_equal)
                        rowsb = twp.tile([128, 4, 256], BF16, tag="rowsb")
                        nc.scalar.activation(rowsb[:], rows4[:],
                                             mybir.ActivationFunctionType.Copy)
                        for ti in range(4):
                            nc.tensor.matmul(out=mg4[:, ti, 0:136],
                                             lhsT=sel4[:, ti, :],
                                             rhs=rowsb[:, ti, 0:136],
                                             start=True, stop=True)
                        nc.vector.tensor_copy(scat[:, :, 0:136],
                                              mg4[:, :, 0:136])
                        nc.gpsimd.dma_scatter_add(
                            ag_dst, scat[:], scidx[:, sl], BLK, BLK, 192)
                    pos += P_g
                ctx_es.__exit__(None, None, None)
                ctx_eb.__exit__(None, None, None)
                # ---- allreduce aggr ----
                nc.gpsimd.collective_compute(
                    "AllReduce", mybir.AluOpType.add,
                    replica_groups=[list(range(NCORES))],
                    ins=[aggr[:, :]], outs=[aggr2[:, :]])
                if debug and l == 0:
                    nc.sync.dma_start(tab_out[:, :], tab_all[:, :])
                    nc.sync.dma_start(ag_out[:, :], aggr2[:, :])
                # ---- update (512-wide) ----
                with tc.tile_pool(name="psupd", bufs=1, space="PSUM") as psu:
                    for (t, c0, c1) in sections:
                        for c in range(c0, c1, 512):
                            w = min(512, c1 - c)
                            nsub = w // 128
                            asb = iop.tile([128, 4, 192], F32, tag="asb")
                            psM = psu.tile([128, 512], F32, tag="psM")
                            psE = psu.tile([HEADS, 512], F32, tag="psE")
                            for j in range(nsub):
                                ci = (c + j * 128) // 128
                                nc.gpsimd.indirect_dma_start(
                                    out=asb[:, j, :], out_offset=None,
                                    in_=aggr2[:, :],
                                    in_offset=bass.IndirectOffsetOnAxis(
                                        ap=updidx[:, ci:ci + 1], axis=0))
                                nc.tensor.transpose(
                                    psM[:, j * 128:(j + 1) * 128],
                                    asb[:, j, 0:128], identf[:])
                                nc.tensor.transpose(
                                    psE[:, j * 128:(j + 1) * 128],
                                    asb[:, j, 128:136], identf[:])
                            den = twp.tile([HEADS, 512], F32, tag="den")
                            nc.vector.tensor_scalar_add(den[:, 0:w], psE[:, 0:w],
                                                        1e-16)
                            rec = twp.tile([HEADS, 512], F32, tag="rec")
                            nc.vector.reciprocal(rec[:, 0:w], den[:, 0:w])
                            psD = psu.tile([128, 512], F32, tag="psD")
                            nc.tensor.matmul(out=psD[:, 0:w], lhsT=headb[:],
                                             rhs=rec[:, 0:w], start=True,
                                             stop=True)
                            dens = twp.tile([128, 512], F32, tag="dens")
                            nc.vector.tensor_copy(dens[:, 0:w], psD[:, 0:w])
                            hsb = twp.tile([128, 512], F32, tag="hsb")
                            nc.vector.tensor_tensor(hsb[:, 0:w], psM[:, 0:w],
                                                    dens[:, 0:w],
                                                    mybir.AluOpType.mult)
                            hgl = twp.tile([128, 512], BF16, tag="hgl")
                            nc.scalar.activation(hgl[:, 0:w], hsb[:, 0:w],
                                                 mybir.ActivationFunctionType.Gelu)
                            psT = psu.tile([128, 512], F32, tag="psT")
                            nc.tensor.matmul(
                                out=psT[:, 0:w],
                                lhsT=awl_sb[:, (l * T + t) * NH:(l * T + t + 1) * NH],
                                rhs=hgl[:, 0:w], start=True, stop=True)
                            tmp = twp.tile([128, 512], F32, tag="tmp")
                            nc.scalar.activation(
                                tmp[:, 0:w], psT[:, 0:w],
                                mybir.ActivationFunctionType.Identity,
                                bias=abl_sb[:, l * T + t:l * T + t + 1])
                            alpha = ALPHAS[l][t]
                            nc.vector.scalar_tensor_tensor(
                                xT[:, c:c + w], xT[:, c:c + w], 1.0 - alpha,
                                tmp[:, 0:w], mybir.AluOpType.mult,
                                mybir.AluOpType.add)

            # ================= output =================
            nc.sync.dma_start(xout[:, :], xT[:])
    nc.compile()
    return nc


ALPHAS = None  # set by kernel() before build (sigmoid(skip), [L][T])


_init_th = None


def _warmup_build():
    """Trigger bass/tile lazy init (cffi cdef parse, isa tables) with a tiny
    throwaway build so the real build doesn't pay the ~0.6s warmup."""
    try:
        nc = bacc.Bacc("TRN2", target_bir_lowering=False, debug=False,
                       num_devices=NCORES)
        a = nc.dram_tensor("a", [128, 128], BF16, kind="ExternalInput").ap()
        o = nc.dram_tensor("o", [128, 128], F32, kind="ExternalOutput").ap()
        with tile.TileContext(nc) as tc:
            with tc.tile_pool(name="w", bufs=1) as w, \
                 tc.tile_pool(name="p", bufs=1, space="PSUM") as p:
                t = w.tile([128, 128], BF16, tag="t")
                nc.sync.dma_start(t[:], a[:, :])
                ps = p.tile([128, 128], F32, tag="ps")
                nc.tensor.matmul(out=ps[:], lhsT=t[:], rhs=t[:], start=True,
                                 stop=True)
                t2 = w.tile([128, 128], F32, tag="t2")
                nc.vector.tensor_copy(t2[:], ps[:])
                nc.sync.dma_start(o[:, :], t2[:])
        nc.compile()
    except Exception:
        pass


def _start_jax_init():
    """Kick off jax/axon backend init + bass warmup in the background."""
    global _init_th
    if _init_th is None:
        import threading

        def work():
            import jax
            jax.devices()

        _init_th = threading.Thread(target=work)
        _init_th.start()


def _prefetch(make_in_maps, out_shapes):
    """Build per-core input arrays and device_put them over the mesh in a
    background thread, overlapping packing + tunnel transfer with NEFF
    build."""
    import threading
    placed = {}

    def work():
        if _init_th is not None:
            _init_th.join()
        import jax
        from jax.sharding import Mesh, PartitionSpec, NamedSharding
        devices = jax.devices()[:NCORES]
        mesh = Mesh(np.asarray(devices), ("core",))
        shd = NamedSharding(mesh, PartitionSpec("core"))
        in_maps = make_in_maps()
        for name in in_maps[0]:
            cat = np.concatenate([np.asarray(in_maps[c][name])
                                  for c in range(NCORES)], axis=0)
            placed[name] = jax.device_put(cat, shd)
        import jax.numpy as jnp
        for name, (shape, dtype) in out_shapes.items():
            full = (NCORES * shape[0], *shape[1:])
            placed['__zero__' + name] = jax.jit(
                lambda full=full, dtype=dtype: jnp.zeros(full, dtype),
                out_shardings=shd)()

    th = threading.Thread(target=work)
    th.start()
    return placed, th


def _run_pre(nc, placed, th):
    """run_bass_via_pjrt (multi-core axon branch) with pre-placed inputs."""
    import jax
    from jax.experimental.shard_map import shard_map
    from jax.sharding import Mesh, PartitionSpec
    from concourse import bass2jax
    bass2jax.install_neuronx_cc_hook()
    assert nc.dbg_addr is None
    partition_name = (nc.partition_id_tensor.name
                      if nc.partition_id_tensor else None)
    in_names, out_names, out_avals = [], [], []
    for alloc in nc.m.functions[0].allocations:
        if not isinstance(alloc, mybir.MemoryLocationSet):
            continue
        name = alloc.memorylocations[0].name
        if alloc.kind == "ExternalInput":
            if name != partition_name:
                in_names.append(name)
        elif alloc.kind == "ExternalOutput":
            assert alloc.tensor_shape is not None and alloc.dtype is not None
            out_names.append(name)
            out_avals.append(jax.core.ShapedArray(
                tuple(alloc.tensor_shape), mybir.dt.np(alloc.dtype)))
    n_params = len(in_names)
    all_names = in_names + out_names
    if partition_name is not None:
        all_names.append(partition_name)
    donate = tuple(range(n_params, n_params + len(out_names)))

    def _body(*args):
        operands = list(args)
        if partition_name is not None:
            operands.append(bass2jax.partition_id_tensor())
        outs = bass2jax._bass_exec_p.bind(
            *operands, out_avals=tuple(out_avals), in_names=tuple(all_names),
            out_names=tuple(out_names), lowering_input_output_aliases=(),
            sim_require_finite=True, sim_require_nnan=True, nc=nc)
        return tuple(outs)

    devices = jax.devices()[:NCORES]
    mesh = Mesh(np.asarray(devices), ("core",))
    P = PartitionSpec
    in_specs = (P("core"),) * (n_params + len(out_names))
    out_specs = (P("core"),) * len(out_names)
    sharded = jax.jit(
        shard_map(_body, mesh=mesh, in_specs=in_specs, out_specs=out_specs,
                  check_rep=False),
        donate_argnums=donate, keep_unused=True)
    _tick('join upload')
    th.join()
    _tick('exec')
    ins = [placed[name] for name in in_names]
    zeros = [placed['__zero__' + name] for name in out_names]
    out_arrs = sharded(*ins, *zeros)
    for a in out_arrs:
        a.block_until_ready()
    _tick('download')
    out = {
        name: np.asarray(out_arrs[i]).reshape(NCORES, *out_avals[i].shape)
        for i, name in enumerate(out_names)
    }
    _tick('download done')
    return out


def wrap16(a):
    return np.ascontiguousarray(a.reshape(-1, 16).T)


def _bf(x):
    return np.ascontiguousarray(x).astype(ml_dtypes.bfloat16)


def kernel(node_feature, adapt_w, adapt_b, k_w, k_b, q_w, q_b, v_w, v_b,
           a_w, a_b, rel_pri, rel_att, rel_msg, skip, rte_tab, rte_w, rte_b,
           node_type, edge_index, edge_type, edge_time):
    global ALPHAS
    _tick('kernel start')
    _start_jax_init()
    node_type = np.asarray(node_type).astype(np.int64)
    src = np.asarray(edge_index[0]).astype(np.int64)
    dst = np.asarray(edge_index[1]).astype(np.int64)
    et = np.asarray(edge_type).astype(np.int64)
    etime = np.asarray(edge_time).astype(np.int64)

    # ---- node partitioning ----
    order = np.argsort(node_type, kind='stable')
    own = [order[c::NCORES] for c in range(NCORES)]
    cnt = np.stack([np.bincount(node_type[o], minlength=T) for o in own])
    tpad = [int(np.ceil((cnt[:, t].max() + 1) / 128) * 128) for t in range(T)]
    LNP = int(sum(tpad))
    offs = np.cumsum([0] + tpad)[:-1]
    sections = [(t, int(offs[t]), int(offs[t] + tpad[t])) for t in range(T)]
    loc2glob = np.full((NCORES, LNP), -1, np.int64)
    for c in range(NCORES):
        o = own[c]
        for t in range(T):
            sec = o[node_type[o] == t]
            loc2glob[c, offs[t]:offs[t] + len(sec)] = sec
    valid = loc2glob >= 0
    l2g0 = np.where(valid, loc2glob, 0)
    row_of = np.empty(N, np.int64)
    for c in range(NCORES):
        row_of[loc2glob[c][valid[c]]] = c * LNP + np.flatnonzero(valid[c])
    RT = NCORES * LNP
    dummy_lo = int(offs[0] + cnt[0, 0])                      # core0 pad row
    dummy_hi = int(7 * LNP + offs[0] + cnt[7, 0])            # core7 pad row
    assert dummy_lo < HALF and HALF <= dummy_hi < RT

    # ---- edge partitioning: 16 groups x 8 cores, dst-sorted, packed ----
    _tick('edge prep start')
    srow = row_of[src]
    drow = row_of[dst]
    st = node_type[src]
    sh = (srow >= HALF).astype(np.int64)
    dh = (drow >= HALF).astype(np.int64)
    garr = et * 4 + sh * 2 + dh

    # per (group, core): edge id lists (dst-sorted), packed into BLK blocks
    # with no dst run straddling a block boundary.
    P_gs = []
    core_slots = [[] for _ in range(NCORES)]   # per core: list of arrays (edge id or -1 pad)
    for g in range(16):
        eg = np.flatnonzero(garr == g)
        eg = eg[np.argsort(drow[eg], kind='stable')]
        nb = len(eg)
        bounds = [nb * c // NCORES for c in range(NCORES + 1)]
        packed = []
        for c in range(NCORES):
            ch = eg[bounds[c]:bounds[c + 1]]
            if len(ch) == 0:
                packed.append(np.zeros(0, np.int64))
                continue
            d = drow[ch]
            runstart = np.flatnonzero(np.r_[True, np.diff(d) != 0])
            runlen = np.diff(np.r_[runstart, len(ch)])
            assert runlen.max() <= 128
            slots = []
            posn = 0
            for rs, rl in zip(runstart, runlen):
                # no dst run may straddle a 128-tile boundary: two leader
                # tokens for one row in a scatter call would race (CCE add)
                rem = 128 - posn % 128
                if rl > rem and posn % 128 != 0:
                    slots.append(np.full(rem, -1, np.int64))
                    posn += rem
                slots.append(ch[rs:rs + rl])
                posn += rl
            arr = np.concatenate(slots)
            packed.append(arr)
        mx = max(len(p) for p in packed)
        P_g = int(np.ceil(mx / BLK) * BLK) if mx else 0
        P_gs.append(P_g)
        for c in range(NCORES):
            p = packed[c]
            core_slots[c].append(np.r_[p, np.full(P_g - len(p), -1, np.int64)])
    EP = int(sum(P_gs))
    NT = EP // 128
    NCH = LNP // 128

    # per-core index arrays
    kvidx_m, rteidx_m, qidx_m, scidx_m, dstid_m = [], [], [], [], []
    for c in range(NCORES):
        eids = np.concatenate(core_slots[c])          # [EP], -1 = pad
        pad = eids < 0
        e0 = np.where(pad, 0, eids)
        kvi = (srow[e0] - HALF * sh[e0]).astype(np.int64)
        rti = (st[e0] * 240 + etime[e0]).astype(np.int64)
        qi = (drow[e0] - HALF * dh[e0]).astype(np.int64)
        kvi[pad] = 0; rti[pad] = 0; qi[pad] = 0
        did = drow[e0].astype(np.float64)
        did[pad] = -1.0
        # leaders: first slot of its dst within each 128-tile
        dd = drow[e0]; dd[pad] = -1
        lead = np.r_[True, dd[1:] != dd[:-1]]
        lead[::128] = True
        lead &= ~pad
        # scatter idx: leader -> real (half-relative), else dummy of the half
        pos = 0
        si = np.empty(EP, np.int64)
        gi = 0
        for (g, P_g) in enumerate(P_gs):
            dhh = g & 1
            dmy = dummy_lo if dhh == 0 else dummy_hi - HALF
            seg = slice(pos, pos + P_g)
            si[seg] = np.where(lead[seg], qi[seg], dmy)
            pos += P_g
        kvidx_m.append(wrap16(kvi.astype(np.int16)))
        rteidx_m.append(wrap16(rti.astype(np.int16)))
        qidx_m.append(wrap16(qi.astype(np.int16)))
        scidx_m.append(wrap16(si.astype(np.int16)))
        dv = np.zeros((128, NT), np.float32)
        dv[np.arange(EP) % 128, np.arange(EP) // 128] = did
        dstid_m.append(dv)
    _tick('edge prep done')

    # ---- weights folding ----
    pri = np.asarray(rel_pri, np.float32)
    ALPHAS = [[float(1.0 / (1.0 + np.exp(-np.asarray(skip, np.float32)[l, t])))
               for t in range(T)] for l in range(L)]

    def bd(mats):  # [H,DK,DK] -> block-diag [NH,NH]
        out = np.zeros((NH, NH), np.float32)
        for h in range(HEADS):
            out[h * DK:(h + 1) * DK, h * DK:(h + 1) * DK] = mats[h]
        return out

    bdatt = np.zeros((L, R, NH, NH), np.float32)
    bdmsg = np.zeros((L, R, NH, NH), np.float32)
    for l in range(L):
        for r in range(R):
            bdatt[l, r] = bd(np.asarray(rel_att[l, r], np.float32)
                             * (pri[l, r][:, None, None] / SQRT_DK))
            bdmsg[l, r] = bd(np.asarray(rel_msg[l, r], np.float32))
    wkvq = np.zeros((L, T, NH, 3 * NH), np.float32)
    bkvq = np.zeros((L * T, 3 * NH), np.float32)
    rte_kv = np.zeros((L, T * 240, 2 * NH), np.float32)
    awl = np.zeros((L, T, NH, NH), np.float32)
    abl = np.zeros((NH, L * T), np.float32)
    for l in range(L):
        kw = np.asarray(k_w[l], np.float32); kb = np.asarray(k_b[l], np.float32)
        qw = np.asarray(q_w[l], np.float32); qb = np.asarray(q_b[l], np.float32)
        vw = np.asarray(v_w[l], np.float32); vb = np.asarray(v_b[l], np.float32)
        rte = (np.asarray(rte_tab[l], np.float32) @ np.asarray(rte_w[l], np.float32)
               + np.asarray(rte_b[l], np.float32))     # [240, NH]
        for t in range(T):
            wkvq[l, t, :, 0:NH] = kw[t]
            wkvq[l, t, :, NH:2 * NH] = vw[t]
            wkvq[l, t, :, 2 * NH:] = qw[t]
            bkvq[l * T + t, 0:NH] = kb[t]
            bkvq[l * T + t, NH:2 * NH] = vb[t]
            bkvq[l * T + t, 2 * NH:] = qb[t]
            rte_kv[l, t * 240:(t + 1) * 240, 0:NH] = rte @ kw[t]
            rte_kv[l, t * 240:(t + 1) * 240, NH:] = rte @ vw[t]
        for t in range(T):
            awl[l, t] = np.asarray(a_w[l, t], np.float32) * ALPHAS[l][t]
            abl[:, l * T + t] = np.asarray(a_b[l, t], np.float32) * ALPHAS[l][t]

    headT = np.zeros((NH, HEADS), np.float32)
    headb = np.zeros((HEADS, NH), np.float32)
    for h in range(HEADS):
        headT[h * DK:(h + 1) * DK, h] = 1.0
        headb[h, h * DK:(h + 1) * DK] = 1.0

    # ---- pack bf16 weights into one blob (order must match device take()) ----
    aw_f = np.asarray(adapt_w, np.float32)
    parts = []
    for t in range(T):
        for k in range(2):
            parts.append(aw_f[t, k * 128:(k + 1) * 128, :])
    for l in range(L):
        for t in range(T):
            parts.append(wkvq[l, t])
    for i in range(L * T):
        parts.append(bkvq[i])
    for l in range(L):
        for r in range(R):
            parts.append(bdatt[l, r])
            parts.append(bdmsg[l, r])
    for l in range(L):
        for t in range(T):
            parts.append(awl[l, t])
    parts.append(np.eye(128, dtype=np.float32))
    parts.append(rte_kv.reshape(-1, 2 * NH))
    blob = np.concatenate([np.ascontiguousarray(p, np.float32).ravel()
                           for p in parts]).astype(ml_dtypes.bfloat16)
    WSHARD = int(np.ceil(len(blob) / (NCORES * 64)) * 64)
    blob = np.r_[blob, np.zeros(NCORES * WSHARD - len(blob),
                                ml_dtypes.bfloat16)]

    # ---- in_maps (built inside the prefetch thread) ----
    nf = np.asarray(node_feature, np.float32)

    def make_in_maps():
        ab_host = np.asarray(adapt_b, np.float32).T.copy()      # [NH, T]
        shared = {
            "ab": ab_host, "abl": abl,
            "headT": headT, "headb": headb,
            "ones1": np.ones((1, NH), ml_dtypes.bfloat16),
            "identf": np.eye(128, dtype=np.float32),
        }
        in_maps = []
        for c in range(NCORES):
            featT = nf[l2g0[c]].T.copy()
            featT[:, ~valid[c]] = 0
            upd = (c * LNP + np.arange(LNP)).reshape(NCH, 128).T.astype(np.int32)
            m = dict(shared)
            m.update({
                "featT": _bf(featT),
                "wblob": blob[c * WSHARD:(c + 1) * WSHARD],
                "kvidx": kvidx_m[c], "rteidx": rteidx_m[c],
                "qidx": qidx_m[c], "scidx": scidx_m[c],
                "dstid": dstid_m[c], "updidx": np.ascontiguousarray(upd),
            })
            in_maps.append(m)
        return in_maps

    _tick('prefetch start')
    placed, th = _prefetch(
        make_in_maps, {"xout": ((NH, LNP), ml_dtypes.bfloat16)})

    # ---- build / compile (overlaps with upload thread) ----
    key = (LNP, tuple(P_gs), WSHARD)
    if key not in _cache:
        _tick('build+compile start')
        _cache[key] = build_neff(LNP, sections, P_gs, NT, NCH, WSHARD)
        _tick('build+compile done')
    nc = _cache[key]

    _tick('launch')
    outs = _run_pre(nc, placed, th)
    _tick('launch done')
    x = np.zeros((N, NH), np.float32)
    for c in range(NCORES):
        xo = outs["xout"][c].astype(np.float32)                  # [NH, LNP]
        x[loc2glob[c][valid[c]]] = xo.T[valid[c]]
    _tick('done')
    return x


# revision 39
# speedup vs baseline: 18.0599x; 15.7508x over previous
"""HGT (heterogeneous graph transformer) on 8 TRN2 NeuronCores.

Single-launch, fully on-device design:
  - Nodes type-sorted, dealt round-robin to 8 cores (SPMD: one NEFF).
  - Device: adapt (tanh per-type linear) -> per layer: per-type K|V|Q table
    matmul for owned nodes -> AllGather table (bf16) -> edge phase on device
    (dma_gather of K/V rows by src, RTE rows by (type,time), Q rows by dst;
    per-relation block-diag matmuls; unstabilized segment softmax via exp +
    scatter-added denominators; selection-matrix merge per 128-edge tile;
    dma_scatter_add into aggr table) -> AllReduce aggr -> normalize / gelu /
    per-type update matmul -> residual mix, all in SBUF-resident xT.
  - Edges sorted by (edge_type, src_half, dst_half, dst_row): 16 groups.
    Halves solve the int16 gather-index range; relation grouping gives
    sliced block-diag matmuls; dst sort + per-tile leader tokens give
    collision-free scatter-adds.
  - Host: only integer index prep + bf16 packing + final transpose.
"""
import sys, math, os, time
sys.path.insert(0, '/opt/trn_rl_repo')
import numpy as np
import ml_dtypes

import concourse.bass as bass
import concourse.tile as tile
from concourse import bacc, mybir
from concourse import bass_utils

BF16 = mybir.dt.bfloat16
F32 = mybir.dt.float32
I16 = mybir.dt.int16
I32 = mybir.dt.int32
NCORES = 8
N, E = 50000, 300000
IN_DIM, NH, HEADS, DK = 256, 128, 8, 16
T, R, L = 3, 4, 2
SQRT_DK = math.sqrt(DK)
HALF = 32768
BLK = 512

_T0 = time.time()
def _tick(msg):
    print(f"[kernel +{time.time()-_T0:7.2f}s] {msg}", file=sys.stderr, flush=True)

_cache = {}


def build_neff(LNP, sections, P_gs, NT, NCH, WSHARD, debug=False):
    """One SPMD NEFF for the whole model.

    sections: list of (type, c0, c1) node ranges (multiples of 128).
    P_gs: list of 16 per-group padded edge counts (multiples of BLK).
    NT = sum(P_gs)//128 tile count; NCH = LNP//128 node chunk count.
    """
    RT = NCORES * LNP
    EP = sum(P_gs)
    nc = bacc.Bacc("TRN2", target_bir_lowering=False, debug=False,
                   num_devices=NCORES)
    # ---- inputs ----
    featT = nc.dram_tensor("featT", [IN_DIM, LNP], BF16, kind="ExternalInput").ap()
    wsh_d = nc.dram_tensor("wblob", [WSHARD], BF16, kind="ExternalInput").ap()
    ab_d = nc.dram_tensor("ab", [NH, T], F32, kind="ExternalInput").ap()
    abl_d = nc.dram_tensor("abl", [NH, L * T], F32, kind="ExternalInput").ap()
    headT_d = nc.dram_tensor("headT", [NH, HEADS], F32, kind="ExternalInput").ap()
    headb_d = nc.dram_tensor("headb", [HEADS, NH], F32, kind="ExternalInput").ap()
    ones1_d = nc.dram_tensor("ones1", [1, NH], BF16, kind="ExternalInput").ap()
    identf_d = nc.dram_tensor("identf", [128, 128], F32, kind="ExternalInput").ap()
    wblob = nc.dram_tensor("wblob_all", [NCORES * WSHARD], BF16,
                           kind="Internal", addr_space="Shared").ap()
    wstage = nc.dram_tensor("wstage", [WSHARD], BF16, kind="Internal").ap()

    def wv(off, rows, cols):
        return wblob[off:off + rows * cols].rearrange("(a b) -> a b", a=rows)
    kvidx_d = nc.dram_tensor("kvidx", [16, EP // 16], I16, kind="ExternalInput").ap()
    rteidx_d = nc.dram_tensor("rteidx", [16, EP // 16], I16, kind="ExternalInput").ap()
    qidx_d = nc.dram_tensor("qidx", [16, EP // 16], I16, kind="ExternalInput").ap()
    scidx_d = nc.dram_tensor("scidx", [16, EP // 16], I16, kind="ExternalInput").ap()
    dstid_d = nc.dram_tensor("dstid", [128, NT], F32, kind="ExternalInput").ap()
    updidx_d = nc.dram_tensor("updidx", [128, NCH], I32, kind="ExternalInput").ap()
    # ---- output ----
    xout = nc.dram_tensor("xout", [NH, LNP], BF16, kind="ExternalOutput").ap()
    if debug:
        xa_out = nc.dram_tensor("xa_out", [NH, LNP], BF16, kind="ExternalOutput").ap()
        tab_out = nc.dram_tensor("tab_out", [NCORES * LNP, 3 * NH], BF16, kind="ExternalOutput").ap()
        ag_out = nc.dram_tensor("ag_out", [NCORES * LNP, 192], F32, kind="ExternalOutput").ap()
    # ---- internal DRAM ----
    tab_own = nc.dram_tensor("tab_own", [LNP, 3 * NH], BF16, kind="Internal").ap()
    tab_all = nc.dram_tensor("tab_all", [RT, 3 * NH], BF16, kind="Internal",
                             addr_space="Shared").ap()
    aggr = nc.dram_tensor("aggr", [RT, 192], F32, kind="Internal").ap()
    aggr2 = nc.dram_tensor("aggr2", [RT, 192], F32, kind="Internal",
                           addr_space="Shared").ap()

    sec_of = {}
    for (t, c0, c1) in sections:
        for c in range(c0, c1, 128):
            sec_of[c] = t

    groups = []  # (et, sh, dh, P_g)
    gi = 0
    for et in range(R):
        for sh in range(2):
            for dh in range(2):
                groups.append((et, sh, dh, P_gs[gi]))
                gi += 1

    with tile.TileContext(nc) as tc:
        with tc.tile_pool(name="persist", bufs=1) as pp, \
             tc.tile_pool(name="wts", bufs=1) as wp, \
             tc.tile_pool(name="io", bufs=3) as iop, \
             tc.tile_pool(name="edge", bufs=2) as ep, \
             tc.tile_pool(name="tilework", bufs=2) as twp:
            # ================= persistent loads =================
            nc.sync.dma_start(wstage[:], wsh_d[:])
            nc.gpsimd.collective_compute(
                "AllGather", mybir.AluOpType.bypass,
                replica_groups=[list(range(NCORES))],
                ins=[wstage[:]], outs=[wblob[:]])
            xT = pp.tile([128, LNP], BF16, tag="xT")
            kvidx = pp.tile([128, EP // 16], I16, tag="kvidx")
            rteidx = pp.tile([128, EP // 16], I16, tag="rteidx")
            qidx = pp.tile([128, EP // 16], I16, tag="qidx")
            scidx = pp.tile([128, EP // 16], I16, tag="scidx")
            for (sb, d) in ((kvidx, kvidx_d), (rteidx, rteidx_d),
                            (qidx, qidx_d), (scidx, scidx_d)):
                for k in range(8):
                    nc.sync.dma_start(sb[16 * k:16 * (k + 1), :], d[:, :])
            dstid = pp.tile([128, NT], F32, tag="dstid")
            nc.sync.dma_start(dstid[:], dstid_d[:, :])
            updidx = pp.tile([128, NCH], I32, tag="updidx")
            nc.sync.dma_start(updidx[:], updidx_d[:, :])
            headT = pp.tile([128, HEADS], F32, tag="headT")
            nc.sync.dma_start(headT[:], headT_d[:, :])
            headb = pp.tile([HEADS, 128], F32, tag="headb")
            nc.sync.dma_start(headb[:], headb_d[:, :])
            ones1 = pp.tile([1, 128], BF16, tag="ones1")
            nc.sync.dma_start(ones1[:], ones1_d[:, :])
            identf = pp.tile([128, 128], F32, tag="identf")
            nc.sync.dma_start(identf[:], identf_d[:, :])
            zero192 = pp.tile([128, 192], F32, tag="zero192")
            nc.vector.memset(zero192[:], 0.0)
            # weights (from allgathered blob; offsets match host pack order)
            cur = [0]

            def take(rows, cols):
                v = wv(cur[0], rows, cols)
                cur[0] += rows * cols
                return v

            aw_sb = wp.tile([128, T * 2 * NH], BF16, tag="aw")
            for t in range(T):
                for k in range(2):
                    nc.sync.dma_start(
                        aw_sb[:, (t * 2 + k) * NH:(t * 2 + k + 1) * NH],
                        take(128, NH))
            wkvq_sb = wp.tile([128, L * T * 3 * NH], BF16, tag="wkvq")
            for l in range(L):
                for t in range(T):
                    nc.sync.dma_start(
                        wkvq_sb[:, (l * T + t) * 3 * NH:(l * T + t + 1) * 3 * NH],
                        take(NH, 3 * NH))
            bkvq_sb = wp.tile([1, L * T * 3 * NH], BF16, tag="bkvq")
            for i in range(L * T):
                nc.sync.dma_start(bkvq_sb[:, i * 3 * NH:(i + 1) * 3 * NH],
                                  take(1, 3 * NH))
            bdatt_sb = wp.tile([128, L * R * NH], BF16, tag="bdatt")
            bdmsg_sb = wp.tile([128, L * R * NH], BF16, tag="bdmsg")
            for l in range(L):
                for r in range(R):
                    nc.sync.dma_start(
                        bdatt_sb[:, (l * R + r) * NH:(l * R + r + 1) * NH],
                        take(NH, NH))
                    nc.sync.dma_start(
                        bdmsg_sb[:, (l * R + r) * NH:(l * R + r + 1) * NH],
                        take(NH, NH))
            awl_sb = wp.tile([128, L * T * NH], BF16, tag="awl")
            for l in range(L):
                for t in range(T):
                    nc.sync.dma_start(
                        awl_sb[:, (l * T + t) * NH:(l * T + t + 1) * NH],
                        take(NH, NH))
            identb = pp.tile([128, 128], BF16, tag="identb")
            nc.sync.dma_start(identb[:], take(128, 128))
            rte_off = [cur[0] + l * (T * 240) * (2 * NH) for l in range(L)]
            rte_view = [wv(rte_off[l], T * 240, 2 * NH) for l in range(L)]
            ab_sb = wp.tile([128, T], F32, tag="ab")
            nc.sync.dma_start(ab_sb[:], ab_d[:, :])
            abl_sb = wp.tile([128, L * T], F32, tag="abl")
            nc.sync.dma_start(abl_sb[:], abl_d[:, :])

            # ================= adapt (512-wide) =================
            with tc.tile_pool(name="psadapt", bufs=2, space="PSUM") as psa:
                for (t, c0, c1) in sections:
                    for c in range(c0, c1, 512):
                        w = min(512, c1 - c)
                        ft = iop.tile([128, 2, 512], BF16, tag="ft")
                        for k in range(2):
                            nc.sync.dma_start(ft[:, k, 0:w],
                                              featT[k * 128:(k + 1) * 128, c:c + w])
                        ps = psa.tile([128, 512], F32, tag="psadapt")
                        for k in range(2):
                            nc.tensor.matmul(
                                out=ps[:, 0:w],
                                lhsT=aw_sb[:, (t * 2 + k) * NH:(t * 2 + k + 1) * NH],
                                rhs=ft[:, k, 0:w], start=(k == 0), stop=(k == 1))
                        nc.scalar.activation(xT[:, c:c + w], ps[:, 0:w],
                                             mybir.ActivationFunctionType.Tanh,
                                             bias=ab_sb[:, t:t + 1])

            if debug:
                nc.sync.dma_start(xa_out[:, :], xT[:])
            # ================= layers =================
            for l in range(L):
                # ---- tab matmul (owned nodes) ----
                with tc.tile_pool(name="pstab", bufs=2, space="PSUM") as pst:
                    for ci in range(LNP // 128):
                        c = ci * 128
                        t = sec_of[c]
                        ps = pst.tile([128, 3 * NH], F32, tag="pstab")
                        nc.tensor.matmul(
                            out=ps[:], lhsT=xT[:, c:c + 128],
                            rhs=wkvq_sb[:, (l * T + t) * 3 * NH:(l * T + t + 1) * 3 * NH],
                            start=True, stop=False)
                        nc.tensor.matmul(
                            out=ps[:], lhsT=ones1[:],
                            rhs=bkvq_sb[:, (l * T + t) * 3 * NH:(l * T + t + 1) * 3 * NH],
                            start=False, stop=True)
                        tsb = iop.tile([128, 3 * NH], BF16, tag="tsb")
                        nc.vector.tensor_copy(tsb[:], ps[:])
                        nc.sync.dma_start(tab_own[c:c + 128, :], tsb[:])
                # ---- allgather tab ----
                nc.gpsimd.collective_compute(
                    "AllGather", mybir.AluOpType.bypass,
                    replica_groups=[list(range(NCORES))],
                    ins=[tab_own[:, :]], outs=[tab_all[:, :]])
                # ---- zero aggr ----
                nc.sync.dma_start(
                    aggr.rearrange("(a b) c -> a b c", a=128),
                    zero192[:].unsqueeze(1).broadcast_to([128, RT // 128, 192]))
                # ---- edge phase ----
                ctx_eb = tc.tile_pool(name="pseb", bufs=1, space="PSUM")
                ctx_es = tc.tile_pool(name="pses", bufs=1, space="PSUM")
                psb = ctx_eb.__enter__()
                pss = ctx_es.__enter__()
                pos = 0
                for (et, sh, dh, P_g) in groups:
                    if P_g == 0:
                        continue
                    kv_src = tab_all[sh * HALF:(HALF if sh == 0 else RT), 0:2 * NH]
                    q_src = tab_all[dh * HALF:(HALF if dh == 0 else RT),
                                    2 * NH:3 * NH]
                    ag_dst = aggr[dh * HALF:(HALF if dh == 0 else RT), :]
                    for b0 in range(0, P_g, BLK):
                        sl = slice((pos + b0) // 16, (pos + b0 + BLK) // 16)
                        kvt = ep.tile([128, 2, BLK], BF16, tag="kvt")
                        nc.gpsimd.dma_gather(kvt[:], kv_src, kvidx[:, sl], BLK,
                                             BLK, 2 * NH, elem_step=3 * NH,
                                             transpose=True)
                        rtt = ep.tile([128, 2, BLK], BF16, tag="rtt")
                        nc.gpsimd.dma_gather(rtt[:], rte_view[l],
                                             rteidx[:, sl], BLK, BLK, 2 * NH,
                                             transpose=True)
                        qt = ep.tile([128, 1, BLK], BF16, tag="qt")
                        nc.gpsimd.dma_gather(qt[:], q_src, qidx[:, sl], BLK,
                                             BLK, NH, elem_step=3 * NH,
                                             transpose=True)
                        nc.vector.tensor_add(kvt[:], kvt[:], rtt[:])
                        k2 = psb.tile([128, BLK], F32, tag="psk2")
                        nc.tensor.matmul(
                            out=k2[:],
                            lhsT=bdatt_sb[:, (l * R + et) * NH:(l * R + et + 1) * NH],
                            rhs=kvt[:, 0, :], start=True, stop=True)
                        v2 = psb.tile([128, BLK], F32, tag="psv2")
                        nc.tensor.matmul(
                            out=v2[:],
                            lhsT=bdmsg_sb[:, (l * R + et) * NH:(l * R + et + 1) * NH],
                            rhs=kvt[:, 1, :], start=True, stop=True)
                        pT = ep.tile([128, BLK], F32, tag="pT")
                        nc.vector.tensor_tensor(pT[:], qt[:, 0, :], k2[:],
                                                mybir.AluOpType.mult)
                        att = psb.tile([HEADS, BLK], F32, tag="psatt")
                        nc.tensor.matmul(out=att[:], lhsT=headT[:], rhs=pT[:],
                                         start=True, stop=True)
                        ex = ep.tile([HEADS, BLK], F32, tag="ex")
                        nc.scalar.activation(ex[:], att[:],
                                             mybir.ActivationFunctionType.Exp)
                        exb = psb.tile([128, BLK], F32, tag="psexb")
                        nc.tensor.matmul(out=exb[:], lhsT=headb[:], rhs=ex[:],
                                         start=True, stop=True)
                        exbs = ep.tile([128, BLK], BF16, tag="exbs")
                        nc.scalar.activation(exbs[:], exb[:],
                                             mybir.ActivationFunctionType.Copy)
                        msgT = ep.tile([128, BLK], BF16, tag="msgT")
                        nc.vector.tensor_tensor(msgT[:], v2[:], exbs[:],
                                                mybir.AluOpType.mult)
                        exs = ep.tile([HEADS, BLK], BF16, tag="exs")
                        nc.scalar.activation(exs[:], ex[:],
                                             mybir.ActivationFunctionType.Copy)
                        scat = ep.tile([128, BLK // 128, 192], F32, tag="scat")
                        tg0 = (pos + b0) // 128
                        # 4 tiles batched: bank-aligned psum slots avoid any
                        # matmul output crossing a 2KB PSUM bank boundary.
                        idT4 = pss.tile([128, 4, 128], F32, tag="psidT")
                        rows4 = pss.tile([128, 4, 256], BF16, tag="psrows")
                        mg4 = pss.tile([128, 4, 256], F32, tag="psmg")
                        for ti in range(4):
                            nc.tensor.transpose(
                                idT4[:, ti, :],
                                dstid[:, tg0 + ti:tg0 + ti + 1].to_broadcast(
                                    [128, 128]),
                                identf[:])
                            nc.tensor.transpose(
                                rows4[:, ti, 0:128],
                                msgT[:, ti * 128:(ti + 1) * 128], identb[:])
                            nc.tensor.transpose(
                                rows4[:, ti, 128:136],
                                exs[:, ti * 128:(ti + 1) * 128],
                                identb[0:HEADS, 0:HEADS])
                        sel4 = twp.tile([128, 4, 128], BF16, tag="selb")
                        nc.vector.tensor_tensor(
                            sel4[:],
                            dstid[:, tg0:tg0 + 4].unsqueeze(2).broadcast_to(
                                [128, 4, 128]),
                            idT4[:], mybir.AluOpType.is_equal)
                        rowsb = twp.tile([128, 4, 256], BF16, tag="rowsb")
                        nc.scalar.activation(rowsb[:], rows4[:],
                                             mybir.ActivationFunctionType.Copy)
                        for ti in range(4):
                            nc.tensor.matmul(out=mg4[:, ti, 0:136],
                                             lhsT=sel4[:, ti, :],
                                             rhs=rowsb[:, ti, 0:136],
                                             start=True, stop=True)
                        nc.vector.tensor_copy(scat[:, :, 0:136],
                                              mg4[:, :, 0:136])
                        nc.gpsimd.dma_scatter_add(
                            ag_dst, scat[:], scidx[:, sl], BLK, BLK, 192)
                    pos += P_g
                ctx_es.__exit__(None, None, None)
                ctx_eb.__exit__(None, None, None)
                # ---- allreduce aggr ----
                nc.gpsimd.collective_compute(
                    "AllReduce", mybir.AluOpType.add,
                    replica_groups=[list(range(NCORES))],
                    ins=[aggr[:, :]], outs=[aggr2[:, :]])
                if debug and l == 0:
                    nc.sync.dma_start(tab_out[:, :], tab_all[:, :])
                    nc.sync.dma_start(ag_out[:, :], aggr2[:, :])
                # ---- update (512-wide) ----
                with tc.tile_pool(name="psupd", bufs=1, space="PSUM") as psu:
                    for (t, c0, c1) in sections:
                        for c in range(c0, c1, 512):
                            w = min(512, c1 - c)
                            nsub = w // 128
                            asb = iop.tile([128, 4, 192], F32, tag="asb")
                            psM = psu.tile([128, 512], F32, tag="psM")
                            psE = psu.tile([HEADS, 512], F32, tag="psE")
                            for j in range(nsub):
                                ci = (c + j * 128) // 128
                                nc.gpsimd.indirect_dma_start(
                                    out=asb[:, j, :], out_offset=None,
                                    in_=aggr2[:, :],
                                    in_offset=bass.IndirectOffsetOnAxis(
                                        ap=updidx[:, ci:ci + 1], axis=0))
                                nc.tensor.transpose(
                                    psM[:, j * 128:(j + 1) * 128],
                                    asb[:, j, 0:128], identf[:])
                                nc.tensor.transpose(
                                    psE[:, j * 128:(j + 1) * 128],
                                    asb[:, j, 128:136], identf[:])
                            den = twp.tile([HEADS, 512], F32, tag="den")
                            nc.vector.tensor_scalar_add(den[:, 0:w], psE[:, 0:w],
                                                        1e-16)
                            rec = twp.tile([HEADS, 512], F32, tag="rec")
                            nc.vector.reciprocal(rec[:, 0:w], den[:, 0:w])
                            psD = psu.tile([128, 512], F32, tag="psD")
                            nc.tensor.matmul(out=psD[:, 0:w], lhsT=headb[:],
                                             rhs=rec[:, 0:w], start=True,
                                             stop=True)
                            dens = twp.tile([128, 512], F32, tag="dens")
                            nc.vector.tensor_copy(dens[:, 0:w], psD[:, 0:w])
                            hsb = twp.tile([128, 512], F32, tag="hsb")
                            nc.vector.tensor_tensor(hsb[:, 0:w], psM[:, 0:w],
                                                    dens[:, 0:w],
                                                    mybir.AluOpType.mult)
                            hgl = twp.tile([128, 512], BF16, tag="hgl")
                            nc.scalar.activation(hgl[:, 0:w], hsb[:, 0:w],
                                                 mybir.ActivationFunctionType.Gelu)
                            psT = psu.tile([128, 512], F32, tag="psT")
                            nc.tensor.matmul(
                                out=psT[:, 0:w],
                                lhsT=awl_sb[:, (l * T + t) * NH:(l * T + t + 1) * NH],
                                rhs=hgl[:, 0:w], start=True, stop=True)
                            tmp = twp.tile([128, 512], F32, tag="tmp")
                            nc.scalar.activation(
                                tmp[:, 0:w], psT[:, 0:w],
                                mybir.ActivationFunctionType.Identity,
                                bias=abl_sb[:, l * T + t:l * T + t + 1])
                            alpha = ALPHAS[l][t]
                            nc.vector.scalar_tensor_tensor(
                                xT[:, c:c + w], xT[:, c:c + w], 1.0 - alpha,
                                tmp[:, 0:w], mybir.AluOpType.mult,
                                mybir.AluOpType.add)

            # ================= output =================
            nc.sync.dma_start(xout[:, :], xT[:])
    nc.compile()
    return nc


ALPHAS = None  # set by kernel() before build (sigmoid(skip), [L][T])


_init_th = None


def _warmup_build():
    """Trigger bass/tile lazy init (cffi cdef parse, isa tables) with a tiny
    throwaway build so the real build doesn't pay the ~0.6s warmup."""
    try:
        nc = bacc.Bacc("TRN2", target_bir_lowering=False, debug=False,
                       num_devices=NCORES)
        a = nc.dram_tensor("a", [128, 128], BF16, kind="ExternalInput").ap()
        o = nc.dram_tensor("o", [128, 128], F32, kind="ExternalOutput").ap()
        with tile.TileContext(nc) as tc:
            with tc.tile_pool(name="w", bufs=1) as w, \
                 tc.tile_pool(name="p", bufs=1, space="PSUM") as p:
                t = w.tile([128, 128], BF16, tag="t")
                nc.sync.dma_start(t[:], a[:, :])
                ps = p.tile([128, 128], F32, tag="ps")
                nc.tensor.matmul(out=ps[:], lhsT=t[:], rhs=t[:], start=True,
                                 stop=True)
                t2 = w.tile([128, 128], F32, tag="t2")
                nc.vector.tensor_copy(t2[:], ps[:])
                nc.sync.dma_start(o[:, :], t2[:])
        nc.compile()
    except Exception:
        pass


def _start_jax_init():
    """Kick off jax/axon backend init + bass warmup in the background."""
    global _init_th
    if _init_th is None:
        import threading

        def work():
            import jax
            jax.devices()

        _init_th = threading.Thread(target=work)
        _init_th.start()


def _prefetch(make_in_maps, out_shapes):
    """Build per-core input arrays and device_put them over the mesh in a
    background thread, overlapping packing + tunnel transfer with NEFF
    build."""
    import threading
    placed = {}

    def work():
        if _init_th is not None:
            _init_th.join()
        import jax
        from jax.sharding import Mesh, PartitionSpec, NamedSharding
        devices = jax.devices()[:NCORES]
        mesh = Mesh(np.asarray(devices), ("core",))
        shd = NamedSharding(mesh, PartitionSpec("core"))
        in_maps = make_in_maps()
        for name in in_maps[0]:
            cat = np.concatenate([np.asarray(in_maps[c][name])
                                  for c in range(NCORES)], axis=0)
            placed[name] = jax.device_put(cat, shd)
        import jax.numpy as jnp
        for name, (shape, dtype) in out_shapes.items():
            full = (NCORES * shape[0], *shape[1:])
            placed['__zero__' + name] = jax.jit(
                lambda full=full, dtype=dtype: jnp.zeros(full, dtype),
                out_shardings=shd)()

    th = threading.Thread(target=work)
    th.start()
    return placed, th


def _run_pre(nc, placed, th):
    """run_bass_via_pjrt (multi-core axon branch) with pre-placed inputs."""
    import jax
    from jax.experimental.shard_map import shard_map
    from jax.sharding import Mesh, PartitionSpec
    from concourse import bass2jax
    bass2jax.install_neuronx_cc_hook()
    assert nc.dbg_addr is None
    partition_name = (nc.partition_id_tensor.name
                      if nc.partition_id_tensor else None)
    in_names, out_names, out_avals = [], [], []
    for alloc in nc.m.functions[0].allocations:
        if not isinstance(alloc, mybir.MemoryLocationSet):
            continue
        name = alloc.memorylocations[0].name
        if alloc.kind == "ExternalInput":
            if name != partition_name:
                in_names.append(name)
        elif alloc.kind == "ExternalOutput":
            assert alloc.tensor_shape is not None and alloc.dtype is not None
            out_names.append(name)
            out_avals.append(jax.core.ShapedArray(
                tuple(alloc.tensor_shape), mybir.dt.np(alloc.dtype)))
    n_params = len(in_names)
    all_names = in_names + out_names
    if partition_name is not None:
        all_names.append(partition_name)
    donate = tuple(range(n_params, n_params + len(out_names)))

    def _body(*args):
        operands = list(args)
        if partition_name is not None:
            operands.append(bass2jax.partition_id_tensor())
        outs = bass2jax._bass_exec_p.bind(
            *operands, out_avals=tuple(out_avals), in_names=tuple(all_names),
            out_names=tuple(out_names), lowering_input_output_aliases=(),
            sim_require_finite=True, sim_require_nnan=True, nc=nc)
        return tuple(outs)

    devices = jax.devices()[:NCORES]
    mesh = Mesh(np.asarray(devices), ("core",))
    P = PartitionSpec
    in_specs = (P("core"),) * (n_params + len(out_names))
    out_specs = (P("core"),) * len(out_names)
    sharded = jax.jit(
        shard_map(_body, mesh=mesh, in_specs=in_specs, out_specs=out_specs,
                  check_rep=False),
        donate_argnums=donate, keep_unused=True)
    _tick('join upload')
    th.join()
    _tick('exec')
    ins = [placed[name] for name in in_names]
    zeros = [placed['__zero__' + name] for name in out_names]
    out_arrs = sharded(*ins, *zeros)
    for a in out_arrs:
        a.block_until_ready()
    _tick('download')
    out = {
        name: np.asarray(out_arrs[i]).reshape(NCORES, *out_avals[i].shape)
        for i, name in enumerate(out_names)
    }
    _tick('download done')
    return out


def wrap16(a):
    return np.ascontiguousarray(a.reshape(-1, 16).T)


def _bf(x):
    return np.ascontiguousarray(x).astype(ml_dtypes.bfloat16)


def kernel(node_feature, adapt_w, adapt_b, k_w, k_b, q_w, q_b, v_w, v_b,
           a_w, a_b, rel_pri, rel_att, rel_msg, skip, rte_tab, rte_w, rte_b,
           node_type, edge_index, edge_type, edge_time):
    global ALPHAS
    _tick('kernel start')
    _start_jax_init()
    node_type = np.asarray(node_type).astype(np.int64)
    src = np.asarray(edge_index[0]).astype(np.int64)
    dst = np.asarray(edge_index[1]).astype(np.int64)
    et = np.asarray(edge_type).astype(np.int64)
    etime = np.asarray(edge_time).astype(np.int64)

    # ---- node partitioning ----
    order = np.argsort(node_type, kind='stable')
    own = [order[c::NCORES] for c in range(NCORES)]
    cnt = np.stack([np.bincount(node_type[o], minlength=T) for o in own])
    tpad = [int(np.ceil((cnt[:, t].max() + 1) / 128) * 128) for t in range(T)]
    LNP = int(sum(tpad))
    offs = np.cumsum([0] + tpad)[:-1]
    sections = [(t, int(offs[t]), int(offs[t] + tpad[t])) for t in range(T)]
    loc2glob = np.full((NCORES, LNP), -1, np.int64)
    for c in range(NCORES):
        o = own[c]
        for t in range(T):
            sec = o[node_type[o] == t]
            loc2glob[c, offs[t]:offs[t] + len(sec)] = sec
    valid = loc2glob >= 0
    l2g0 = np.where(valid, loc2glob, 0)
    row_of = np.empty(N, np.int64)
    for c in range(NCORES):
        row_of[loc2glob[c][valid[c]]] = c * LNP + np.flatnonzero(valid[c])
    RT = NCORES * LNP
    dummy_lo = int(offs[0] + cnt[0, 0])                      # core0 pad row
    dummy_hi = int(7 * LNP + offs[0] + cnt[7, 0])            # core7 pad row
    assert dummy_lo < HALF and HALF <= dummy_hi < RT

    # ---- edge partitioning: 16 groups x 8 cores, dst-sorted, packed ----
    _tick('edge prep start')
    srow = row_of[src]
    drow = row_of[dst]
    st = node_type[src]
    sh = (srow >= HALF).astype(np.int64)
    dh = (drow >= HALF).astype(np.int64)
    garr = et * 4 + sh * 2 + dh

    # per (group, core): edge id lists (dst-sorted), packed into BLK blocks
    # with no dst run straddling a block boundary.
    P_gs = []
    core_slots = [[] for _ in range(NCORES)]   # per core: list of arrays (edge id or -1 pad)
    for g in range(16):
        eg = np.flatnonzero(garr == g)
        eg = eg[np.argsort(drow[eg], kind='stable')]
        nb = len(eg)
        bounds = [nb * c // NCORES for c in range(NCORES + 1)]
        packed = []
        for c in range(NCORES):
            ch = eg[bounds[c]:bounds[c + 1]]
            if len(ch) == 0:
                packed.append(np.zeros(0, np.int64))
                continue
            d = drow[ch]
            runstart = np.flatnonzero(np.r_[True, np.diff(d) != 0])
            runlen = np.diff(np.r_[runstart, len(ch)])
            assert runlen.max() <= 128
            # no dst run may straddle a 128-tile boundary: two leader
            # tokens for one row in a scatter call would race (CCE add).
            # Greedy first-fit into 128-tiles, vectorized per tile.
            nruns = len(runlen)
            cum = np.cumsum(runlen)
            starts = np.empty(nruns, np.int64)
            i = 0
            posn = 0
            while i < nruns:
                rem = 128 - posn % 128
                j = np.searchsorted(cum, (cum[i - 1] if i else 0) + rem,
                                    side='right')
                if j == i:
                    posn += rem
                    continue
                starts[i:j] = posn + cum[i:j] - runlen[i:j] - \
                    (cum[i - 1] if i else 0)
                posn = starts[j - 1] + runlen[j - 1]
                i = j
            arr = np.full(int(posn), -1, np.int64)
            within = np.arange(len(ch)) - np.repeat(runstart, runlen)
            arr[np.repeat(starts, runlen) + within] = ch
            packed.append(arr)
        mx = max(len(p) for p in packed)
        P_g = int(np.ceil(mx / BLK) * BLK) if mx else 0
        P_gs.append(P_g)
        for c in range(NCORES):
            p = packed[c]
            core_slots[c].append(np.r_[p, np.full(P_g - len(p), -1, np.int64)])
    EP = int(sum(P_gs))
    NT = EP // 128
    NCH = LNP // 128

    # per-core index arrays
    kvidx_m, rteidx_m, qidx_m, scidx_m, dstid_m = [], [], [], [], []
    for c in range(NCORES):
        eids = np.concatenate(core_slots[c])          # [EP], -1 = pad
        pad = eids < 0
        e0 = np.where(pad, 0, eids)
        kvi = (srow[e0] - HALF * sh[e0]).astype(np.int64)
        rti = (st[e0] * 240 + etime[e0]).astype(np.int64)
        qi = (drow[e0] - HALF * dh[e0]).astype(np.int64)
        kvi[pad] = 0; rti[pad] = 0; qi[pad] = 0
        did = drow[e0].astype(np.float64)
        did[pad] = -1.0
        # leaders: first slot of its dst within each 128-tile
        dd = drow[e0]; dd[pad] = -1
        lead = np.r_[True, dd[1:] != dd[:-1]]
        lead[::128] = True
        lead &= ~pad
        # scatter idx: leader -> real (half-relative), else dummy of the half
        pos = 0
        si = np.empty(EP, np.int64)
        gi = 0
        for (g, P_g) in enumerate(P_gs):
            dhh = g & 1
            dmy = dummy_lo if dhh == 0 else dummy_hi - HALF
            seg = slice(pos, pos + P_g)
            si[seg] = np.where(lead[seg], qi[seg], dmy)
            pos += P_g
        kvidx_m.append(wrap16(kvi.astype(np.int16)))
        rteidx_m.append(wrap16(rti.astype(np.int16)))
        qidx_m.append(wrap16(qi.astype(np.int16)))
        scidx_m.append(wrap16(si.astype(np.int16)))
        dv = np.zeros((128, NT), np.float32)
        dv[np.arange(EP) % 128, np.arange(EP) // 128] = did
        dstid_m.append(dv)
    _tick('edge prep done')

    # ---- weights folding ----
    pri = np.asarray(rel_pri, np.float32)
    ALPHAS = [[float(1.0 / (1.0 + np.exp(-np.asarray(skip, np.float32)[l, t])))
               for t in range(T)] for l in range(L)]

    def bd(mats):  # [H,DK,DK] -> block-diag [NH,NH]
        out = np.zeros((NH, NH), np.float32)
        for h in range(HEADS):
            out[h * DK:(h + 1) * DK, h * DK:(h + 1) * DK] = mats[h]
        return out

    bdatt = np.zeros((L, R, NH, NH), np.float32)
    bdmsg = np.zeros((L, R, NH, NH), np.float32)
    for l in range(L):
        for r in range(R):
            bdatt[l, r] = bd(np.asarray(rel_att[l, r], np.float32)
                             * (pri[l, r][:, None, None] / SQRT_DK))
            bdmsg[l, r] = bd(np.asarray(rel_msg[l, r], np.float32))
    wkvq = np.zeros((L, T, NH, 3 * NH), np.float32)
    bkvq = np.zeros((L * T, 3 * NH), np.float32)
    rte_kv = np.zeros((L, T * 240, 2 * NH), np.float32)
    awl = np.zeros((L, T, NH, NH), np.float32)
    abl = np.zeros((NH, L * T), np.float32)
    for l in range(L):
        kw = np.asarray(k_w[l], np.float32); kb = np.asarray(k_b[l], np.float32)
        qw = np.asarray(q_w[l], np.float32); qb = np.asarray(q_b[l], np.float32)
        vw = np.asarray(v_w[l], np.float32); vb = np.asarray(v_b[l], np.float32)
        rte = (np.asarray(rte_tab[l], np.float32) @ np.asarray(rte_w[l], np.float32)
               + np.asarray(rte_b[l], np.float32))     # [240, NH]
        for t in range(T):
            wkvq[l, t, :, 0:NH] = kw[t]
            wkvq[l, t, :, NH:2 * NH] = vw[t]
            wkvq[l, t, :, 2 * NH:] = qw[t]
            bkvq[l * T + t, 0:NH] = kb[t]
            bkvq[l * T + t, NH:2 * NH] = vb[t]
            bkvq[l * T + t, 2 * NH:] = qb[t]
            rte_kv[l, t * 240:(t + 1) * 240, 0:NH] = rte @ kw[t]
            rte_kv[l, t * 240:(t + 1) * 240, NH:] = rte @ vw[t]
        for t in range(T):
            awl[l, t] = np.asarray(a_w[l, t], np.float32) * ALPHAS[l][t]
            abl[:, l * T + t] = np.asarray(a_b[l, t], np.float32) * ALPHAS[l][t]

    headT = np.zeros((NH, HEADS), np.float32)
    headb = np.zeros((HEADS, NH), np.float32)
    for h in range(HEADS):
        headT[h * DK:(h + 1) * DK, h] = 1.0
        headb[h, h * DK:(h + 1) * DK] = 1.0

    # ---- pack bf16 weights into one blob (order must match device take()) ----
    aw_f = np.asarray(adapt_w, np.float32)
    parts = []
    for t in range(T):
        for k in range(2):
            parts.append(aw_f[t, k * 128:(k + 1) * 128, :])
    for l in range(L):
        for t in range(T):
            parts.append(wkvq[l, t])
    for i in range(L * T):
        parts.append(bkvq[i])
    for l in range(L):
        for r in range(R):
            parts.append(bdatt[l, r])
            parts.append(bdmsg[l, r])
    for l in range(L):
        for t in range(T):
            parts.append(awl[l, t])
    parts.append(np.eye(128, dtype=np.float32))
    parts.append(rte_kv.reshape(-1, 2 * NH))
    blob = np.concatenate([np.ascontiguousarray(p, np.float32).ravel()
                           for p in parts]).astype(ml_dtypes.bfloat16)
    WSHARD = int(np.ceil(len(blob) / (NCORES * 64)) * 64)
    blob = np.r_[blob, np.zeros(NCORES * WSHARD - len(blob),
                                ml_dtypes.bfloat16)]

    # ---- in_maps (built inside the prefetch thread) ----
    nf = np.asarray(node_feature, np.float32)

    def make_in_maps():
        ab_host = np.asarray(adapt_b, np.float32).T.copy()      # [NH, T]
        shared = {
            "ab": ab_host, "abl": abl,
            "headT": headT, "headb": headb,
            "ones1": np.ones((1, NH), ml_dtypes.bfloat16),
            "identf": np.eye(128, dtype=np.float32),
        }
        in_maps = []
        for c in range(NCORES):
            featT = nf[l2g0[c]].T.copy()
            featT[:, ~valid[c]] = 0
            upd = (c * LNP + np.arange(LNP)).reshape(NCH, 128).T.astype(np.int32)
            m = dict(shared)
            m.update({
                "featT": _bf(featT),
                "wblob": blob[c * WSHARD:(c + 1) * WSHARD],
                "kvidx": kvidx_m[c], "rteidx": rteidx_m[c],
                "qidx": qidx_m[c], "scidx": scidx_m[c],
                "dstid": dstid_m[c], "updidx": np.ascontiguousarray(upd),
            })
            in_maps.append(m)
        return in_maps

    _tick('prefetch start')
    placed, th = _prefetch(
        make_in_maps, {"xout": ((NH, LNP), ml_dtypes.bfloat16)})

    # ---- build / compile (overlaps with upload thread) ----
    key = (LNP, tuple(P_gs), WSHARD)
    if key not in _cache:
        _tick('build+compile start')
        _cache[key] = build_neff(LNP, sections, P_gs, NT, NCH, WSHARD)
        _tick('build+compile done')
    nc = _cache[key]

    _tick('launch')
    outs = _run_pre(nc, placed, th)
    _tick('launch done')
    x = np.zeros((N, NH), np.float32)
    for c in range(NCORES):
        xo = outs["xout"][c].astype(np.float32)                  # [NH, LNP]
        x[loc2glob[c][valid[c]]] = xo.T[valid[c]]
    _tick('done')
    return x
